# revision 1
# baseline (speedup 1.0000x reference)
"""Trainium2 Bass kernel for nn_AqSolModel (GNN message passing), 8 NeuronCores.

Strategy:
- Node-sharded: core c owns 6250 nodes, permuted into 49 blocks x 128 slots
  (bin-packed so per-block edge counts fit fixed tile budgets TL/TH).
- Per layer: AllGather bf16 activations into a DRAM pool; dma_gather fetches
  per-edge source rows (two base-offset streams for int16 index range);
  segment-sum via matmuls against host-built 0/1 selection tiles M
  (PSUM-accumulated per dst block) + identity matmul for the self loop.
- BatchNorms folded on host: BN_in pushed past aggregation
  (agg' = gin*agg + bin*deg), BN_out folded into second dense weights/bias.
- Dense layers alternate matmul orientation so no transposes are needed.
- Mean-pool via per-block selection matmul into a per-core graph window;
  windows AllGathered and reconstructed on every core; small dense head runs
  redundantly on all cores; core 0's output is returned.

All index/selection data is computed on the host from edge_index/batch at
build time (the Bass graph is compiled after seeing the inputs), but all
feature compute runs on device.
"""
import sys
sys.path.insert(0, "/opt/trn_rl_repo")

import numpy as np
import ml_dtypes

BF16 = ml_dtypes.bfloat16

N_NODES, N_EDGES, N_FEAT, HID, HID1, N_GRAPHS, N_CONV, N_LIN = (
    50000, 150000, 128, 512, 320, 2048, 4, 3)
EPS = 1e-5
NC_ = 8
SHARD = N_NODES // NC_          # 6250
BLKS = 49
SLOTS = BLKS * 128              # 6272
POOL = SLOTS * NC_              # 50176
BASE_B = POOL - 32768           # 17408; stream B idx = row-BASE_B <= 32767
OVL_LO, OVL_HI = BASE_B, 32768  # rows assignable to either stream
PG = 384                        # pooling window width (3*128)
GRP = 4                         # blocks per gather/dense group
F1P = 384                       # HID1 padded to 3*128

# ---------------------------------------------------------------- host planning

def _pack_blocks3(degA, degB, degF, capA, capB):
    """Assign nodes to blocks s.t. per block: sum(A) <= capA, sum(B) <= capB,
    sum(A+B+F) <= capA+capB. Greedy FFD on total degree."""
    n = len(degA)
    tot = degA + degB + degF
    order = np.argsort(-tot)
    blk_cnt = np.zeros(BLKS, np.int32)
    bA = np.zeros(BLKS, np.int64)
    bB = np.zeros(BLKS, np.int64)
    bT = np.zeros(BLKS, np.int64)
    assign = np.full(n, -1, np.int32)
    for node in order:
        a, b2, t = degA[node], degB[node], tot[node]
        ok = ((blk_cnt < 128) & (bA + a <= capA) & (bB + b2 <= capB)
              & (bT + t <= capA + capB))
        if not ok.any():
            return None
        cand = np.nonzero(ok)[0]
        j = cand[np.argmin(bT[cand])]  # least-loaded: balances degree
        assign[node] = j
        blk_cnt[j] += 1
        bA[j] += a
        bB[j] += b2
        bT[j] += t
    slot = np.full(n, -1, np.int32)
    nxt = np.zeros(BLKS, np.int32)
    for node in range(n):
        j = assign[node]
        slot[node] = j * 128 + nxt[j]
        nxt[j] += 1
    return slot


def _pack_blocks(deg_low, deg_high, tl_cap, th_cap):
    n = len(deg_low)
    order = np.argsort(-(deg_low + deg_high))
    blk_cnt = np.zeros(BLKS, np.int32)
    blk_low = np.zeros(BLKS, np.int64)
    blk_high = np.zeros(BLKS, np.int64)
    assign = np.full(n, -1, np.int32)
    for node in order:
        dl, dh = deg_low[node], deg_high[node]
        ok = (blk_cnt < 128) & (blk_low + dl <= tl_cap) & (blk_high + dh <= th_cap)
        if not ok.any():
            return None
        cand = np.nonzero(ok)[0]
        b = cand[np.argmin((tl_cap - blk_low[cand] - dl)
                           + (th_cap - blk_high[cand] - dh))]
        assign[node] = b
        blk_cnt[b] += 1
        blk_low[b] += dl
        blk_high[b] += dh
    slot = np.full(n, -1, np.int32)
    nxt = np.zeros(BLKS, np.int32)
    for node in range(n):
        b = assign[node]
        slot[node] = b * 128 + nxt[b]
        nxt[b] += 1
    return slot


def build_plan(edge_index, batch):
    src = edge_index[0].astype(np.int64)
    dst = edge_index[1].astype(np.int64)
    core_of = np.minimum(np.arange(N_NODES) // SHARD, NC_ - 1)

    # stream A: pool rows < 32768 (base 0); stream B: rows >= BASE_B (base
    # BASE_B). Rows in [BASE_B, 32768) may go to either stream; the packer
    # only needs per-node FIXED stream degrees, flex edges balance later.
    # pool_row depends on slot assignment; the A/B classification of a source
    # depends on its pool row. Fixed point: pool rows of core c span
    # [6272c, 6272(c+1)). Cores 0-1 fully < BASE_B (fixed A); core 2 rows
    # 12544..18815 straddle BASE_B=17408 -> within [BASE_B,32768) = flex or
    # fixed-A; cores 0-4 all < 32768... classification by ROW RANGE:
    #   row < 17408 (cores 0-2 partial): fixed A
    #   17408 <= row < 32768 (core 2 part, 3, 4, 5 part): flex
    #   row >= 32768 (cores 5 part, 6, 7): fixed B
    # Use the CORE to bound conservatively (independent of slot):
    #   core <= 1: all rows < 12544 -> fixed A
    #   core == 2: rows in [12544, 18816): may be < or >= 17408 -> treat flex
    #   cores 3,4: rows in [18816, 31360) -> flex
    #   core == 5: rows in [31360, 37632): straddles 32768 -> treat fixed B
    #              (rows < 32768 could be flex; conservative B is fine)
    #   cores 6,7: fixed B
    # Core-based (non-circular): cores 0-2 rows < 18816 < 32768 -> valid in A;
    # cores 5-7 rows >= 31360 >= BASE_B -> valid in B; cores 3-4 rows in
    # [18816, 31360) are valid in EITHER stream -> flex.
    src_core = core_of[src]
    fixA = src_core <= 2
    fixB = src_core >= 5
    flex = ~fixA & ~fixB

    degA_n = np.bincount(dst[fixA], minlength=N_NODES)
    degB_n = np.bincount(dst[fixB], minlength=N_NODES)
    degF_n = np.bincount(dst[flex], minlength=N_NODES)

    slot_of = np.zeros(N_NODES, np.int64)
    TL, TH = 2, 2
    for c in range(NC_):
        nodes = np.arange(c * SHARD, (c + 1) * SHARD)
        while True:
            s = _pack_blocks3(degA_n[nodes], degB_n[nodes], degF_n[nodes],
                              TL * 128, TH * 128)
            if s is not None:
                break
            if TL <= TH:
                TL += 1
            else:
                TH += 1
        # may repack later cores at grown caps; earlier cores already fit
    for c in range(NC_):
        nodes = np.arange(c * SHARD, (c + 1) * SHARD)
        slot_of[nodes] = _pack_blocks3(degA_n[nodes], degB_n[nodes],
                                       degF_n[nodes], TL * 128, TH * 128)

    pool_row = core_of * 6272 + slot_of
    NT = TL + TH

    dst_core = core_of[dst]
    dst_slot = slot_of[dst]
    dst_blk = dst_slot // 128
    dst_col = dst_slot % 128

    idx_all = np.zeros((NC_, BLKS, NT, 128), np.int16)
    m_all = np.zeros((NC_, BLKS, NT, 128, 128), np.float32)
    must_A = fixA
    must_B = fixB
    for c in range(NC_):
        sel = dst_core == c
        e_idx = np.nonzero(sel)[0]
        b_of = dst_blk[e_idx]
        order = np.argsort(b_of, kind="stable")
        e_idx = e_idx[order]
        b_of = b_of[order]
        bounds = np.searchsorted(b_of, np.arange(BLKS + 1))
        for b in range(BLKS):
            es = e_idx[bounds[b]:bounds[b + 1]]
            a_es = es[must_A[es]]
            b_es = es[must_B[es]]
            f_es = es[~must_A[es] & ~must_B[es]]
            capA, capB = TL * 128, TH * 128
            roomA = capA - len(a_es)
            nA = min(len(f_es), max(0, roomA))
            # also respect capB for the rest
            assert len(f_es) - nA <= capB - len(b_es), (c, b)
            a_full = np.concatenate([a_es, f_es[:nA]])
            b_full = np.concatenate([b_es, f_es[nA:]])
            for strm, eset, base, t0 in ((0, a_full, 0, 0),
                                         (1, b_full, BASE_B, TL)):
                rel = pool_row[src[eset]] - base
                t = t0 + np.arange(len(eset)) // 128
                r = np.arange(len(eset)) % 128
                idx_all[c, b, t, r] = rel.astype(np.int16)
                m_all[c, b, t, r, dst_col[eset]] = 1.0

    deg = np.bincount(dst, minlength=N_NODES).astype(np.float32) + 1.0
    deg_slots = np.zeros((NC_, SLOTS), np.float32)
    deg_slots[core_of, slot_of] = deg

    # pooling
    cnt = np.bincount(batch, minlength=N_GRAPHS).astype(np.float32)
    inv_cnt = (1.0 / np.maximum(cnt, 1.0)).astype(np.float32)
    g_of = batch.astype(np.int64)
    wbase = np.zeros(NC_, np.int32)
    mpool = np.zeros((NC_, BLKS, 128, PG), np.float32)
    for c in range(NC_):
        nodes = np.arange(c * SHARD, (c + 1) * SHARD)
        gmin, gmax = g_of[nodes].min(), g_of[nodes].max()
        wb = min(max(0, (gmin + gmax + 1) // 2 - PG // 2), N_GRAPHS - PG)
        wb = min(wb, gmin)
        wb = max(wb, gmax - PG + 1)
        assert wb >= 0 and wb + PG <= N_GRAPHS and gmin >= wb and gmax < wb + PG, \
            (c, gmin, gmax, wb)
        wbase[c] = wb
        cols = slot_of[nodes] % 128
        blks = slot_of[nodes] // 128
        mpool[c, blks, cols, g_of[nodes] - wb] = inv_cnt[g_of[nodes]]

    return dict(slot_of=slot_of, core_of=core_of, pool_row=pool_row,
                TL=TL, TH=TH, idx=idx_all, M=m_all, deg=deg_slots,
                mpool=mpool, wbase=wbase)


def fold_params(p):
    out = []
    for l in range(5):
        if l == 0:
            ing, inb, inm, inv = p['in_g1'], p['in_b1'], p['in_m1'], p['in_v1']
            wa, ba, wb, bb = p['w1a'], p['b1a'], p['w1b'], p['b1b']
            og, ob, om, ov = p['out_g1'], p['out_b1'], p['out_m1'], p['out_v1']
        else:
            i = l - 1
            ing, inb, inm, inv = (p['cin_g'][i], p['cin_b'][i],
                                  p['cin_m'][i], p['cin_v'][i])
            wa, ba, wb, bb = p['cwA'][i], p['cbA'][i], p['cwB'][i], p['cbB'][i]
            og, ob, om, ov = (p['cout_g'][i], p['cout_b'][i],
                              p['cout_m'][i], p['cout_v'][i])
        gin = np.asarray(ing / np.sqrt(inv + EPS), np.float32)
        bin_ = np.asarray(inb - inm * gin, np.float32)
        gout = np.asarray(og / np.sqrt(ov + EPS), np.float32)
        bout = np.asarray(ob - om * gout, np.float32)
        WB = np.asarray(wb, np.float32) * gout[None, :]
        bB = np.asarray(bb, np.float32) * gout + bout
        out.append(dict(gin=gin, bin=bin_,
                        WA=np.asarray(wa, np.float32),
                        bA=np.asarray(ba, np.float32), WB=WB, bB=bB))
    return out


# ---------------------------------------------------------------- device build

def build_device(TL, TH):
    """Build the Bacc graph (shapes only; all data arrives via in_maps)."""
    from concourse import bass, bacc, mybir, tile

    NT = TL + TH
    dt = mybir.dt
    nc = bacc.Bacc("TRN2", target_bir_lowering=False, debug=False,
                   enable_asserts=False, num_devices=NC_,
                   num_swdge_queues=4)

    def inp(name, shape, dtype):
        return nc.dram_tensor(name, shape, dtype, kind="ExternalInput")

    x_in = inp("x", [SLOTS, N_FEAT], dt.bfloat16)
    gx_in = inp("gx", [128, BLKS * 4 * 128], dt.bfloat16)
    gl_in = inp("gl", [128, BLKS * TL * 8], dt.int16)
    gh_in = inp("gh", [128, BLKS * TH * 8], dt.int16)
    m_in = inp("m", [128, BLKS * NT * 128], dt.bfloat16)
    m8_in = inp("m8", [128, BLKS * NT * 128], dt.float8e4)
    mp_in = inp("mp", [128, BLKS * PG], dt.bfloat16)
    deg_in = inp("deg", [128, SLOTS], dt.bfloat16)
    pvec_in = inp("pvec", [128, 80], dt.float32)
    brow_in = inp("brow", [1, 5 * 512], dt.bfloat16)
    browb_in = inp("browb", [128, 4 * 512], dt.bfloat16)
    ident_in = inp("ident", [128, 128], dt.bfloat16)
    ones_in = inp("ones", [1, 128], dt.bfloat16)
    wa0_in = inp("wa0", [128, F1P], dt.bfloat16)
    wb0_in = inp("wb0", [128, 3 * 512], dt.bfloat16)
    wa_in = inp("wa", [4, 128, 4 * 512], dt.bfloat16)
    wb_in = inp("wb", [4, 128, 4 * 512], dt.bfloat16)
    lw_in = inp("lw", [3, 128, 4 * 512], dt.bfloat16)
    fw_in = inp("fw", [128, 4], dt.bfloat16)
    out_ext = nc.dram_tensor("out", [N_GRAPHS, 1], dt.float32,
                             kind="ExternalOutput")

    # group structure: 12 groups of 4 blocks + 1 group of 1 block
    groups = [list(range(g * GRP, min((g + 1) * GRP, BLKS)))
              for g in range((BLKS + GRP - 1) // GRP)]

    # pvec column map
    def PV(l, kind, chunk):
        base = {"gin": 0, "bin": 4, "bA": 8}[kind]
        return 12 * l + base + chunk
    PV_LB = lambda l, chunk: 60 + 4 * l + chunk

    with tile.TileContext(nc) as tc:
        import contextlib
        ctx = contextlib.ExitStack()
        with ctx:
            dram = ctx.enter_context(tc.tile_pool(name="dram", bufs=1,
                                                  space="DRAM"))
            const = ctx.enter_context(tc.tile_pool(name="const", bufs=1))
            work = ctx.enter_context(tc.tile_pool(name="work", bufs=1))

            # DRAM: activation pools (ping-pong) + bounces
            pools_u = [dram.tile([POOL, HID], dt.float8e4, addr_space="Shared",
                                 name=f"pool_u{i}") for i in range(4)]
            bounces_u = [dram.tile([SLOTS, HID], dt.float8e4,
                                   name=f"bounce_u{i}") for i in range(4)]
            win_bounce = dram.tile([4 * 128, PG], dt.bfloat16)
            wins_all = dram.tile([NC_ * 4 * 128, PG], dt.bfloat16,
                                 addr_space="Shared")

            # persistent SBUF
            gl_sb = const.tile([128, BLKS * TL * 8], dt.int16)
            gh_sb = const.tile([128, BLKS * TH * 8], dt.int16)
            ux = const.tile([128, BLKS * N_FEAT], dt.bfloat16)
            u_loc = const.tile([128, BLKS * HID], dt.bfloat16)
            deg_sb = const.tile([128, SLOTS], dt.bfloat16)
            pvec = const.tile([128, 80], dt.float32)
            brow = const.tile([1, 5 * 512], dt.bfloat16)
            browb = const.tile([128, 4 * 512], dt.bfloat16)
            ident = const.tile([128, 128], dt.bfloat16)
            ones = const.tile([1, 128], dt.bfloat16)
            wa0 = const.tile([128, F1P], dt.bfloat16)
            wb0 = const.tile([128, 3 * 512], dt.bfloat16)
            wa_sb = [const.tile([128, 4 * 512], dt.bfloat16, name=f"wa{i}")
                     for i in range(4)]
            wb_sb = [const.tile([128, 4 * 512], dt.bfloat16, name=f"wb{i}")
                     for i in range(4)]
            lw_sb = [const.tile([128, 4 * 512], dt.bfloat16, name=f"lwt{i}")
                     for i in range(3)]
            fw_sb = const.tile([128, 4], dt.bfloat16)

            nc.sync.dma_start(out=gl_sb[:], in_=gl_in[:])
            nc.sync.dma_start(out=gh_sb[:], in_=gh_in[:])
            nc.sync.dma_start(out=deg_sb[:], in_=deg_in[:])
            nc.sync.dma_start(out=pvec[:], in_=pvec_in[:])
            nc.sync.dma_start(out=brow[:], in_=brow_in[:])
            nc.sync.dma_start(out=browb[:], in_=browb_in[:])
            nc.sync.dma_start(out=ident[:], in_=ident_in[:])
            nc.sync.dma_start(out=ones[:], in_=ones_in[:])
            nc.sync.dma_start(out=wa0[:], in_=wa0_in[:])
            nc.sync.dma_start(out=wb0[:], in_=wb0_in[:])
            for i in range(4):
                nc.sync.dma_start(out=wa_sb[i][:], in_=wa_in[i])
                nc.sync.dma_start(out=wb_sb[i][:], in_=wb_in[i])
            for i in range(3):
                nc.sync.dma_start(out=lw_sb[i][:], in_=lw_in[i])
            nc.sync.dma_start(out=fw_sb[:], in_=fw_in[:])

            # x: input -> SBUF (selfloop) and -> bounce -> AllGather pool_x
            nc.sync.dma_start(
                out=ux[:].rearrange("p (b f) -> p b f", b=BLKS),
                in_=x_in[:].rearrange("(b p) f -> p b f", p=128))

            conv_ctx = contextlib.ExitStack()
            gpool = conv_ctx.enter_context(tc.tile_pool(name="gpool", bufs=2))
            mpoolp = conv_ctx.enter_context(tc.tile_pool(name="mpoolp", bufs=2))
            aggp = conv_ctx.enter_context(tc.tile_pool(name="aggp", bufs=8))
            h1p = conv_ctx.enter_context(tc.tile_pool(name="h1p", bufs=8))
            psA = conv_ctx.enter_context(tc.tile_pool(name="psA", bufs=4,
                                                      space="PSUM"))
            psB = conv_ctx.enter_context(tc.tile_pool(name="psB", bufs=2,
                                                      space="PSUM"))
            psC = conv_ctx.enter_context(tc.tile_pool(name="psC", bufs=2,
                                                      space="PSUM"))

            def conv_layer(l, src_pool, F_in, u_src, dst_pool, dst_bounce):
                """One sumconv layer. u_src: SBUF tile of local activations
                (selfloop source, layout [128, BLKS*F_in]). Writes new u into
                u_loc and (if dst_pool) DMA+AllGather to it."""
                FC = F_in // 128           # feature chunks of input
                F1C = 3 if l == 0 else 4   # chunks of hidden1 dim
                wa_t = wa0 if l == 0 else wa_sb[l - 1]
                wb_t = wb0 if l == 0 else wb_sb[l - 1]
                f1w = F1P if l == 0 else 512  # wa col width per fc chunk
                gdt = dt.bfloat16 if l == 0 else dt.float8e4
                mt_in = m_in if l == 0 else m8_in
                pair_cache = {}
                for gi, blks in enumerate(groups):
                    nb = len(blks)
                    b0 = blks[0]
                    if gi in pair_cache:
                        g_l, g_h, toff = pair_cache.pop(gi)
                    else:
                        toff = 0
                        span = nb
                        if (nb == GRP and gi + 1 < len(groups)
                                and len(groups[gi + 1]) == GRP):
                            span = 2 * GRP
                        g_l = gpool.tile([128, span * TL, F_in], gdt,
                                         tag="gl", bufs=2)
                        g_h = gpool.tile([128, span * TH, F_in], gdt,
                                         tag="gh", bufs=2)

                        def gather_one(gt, src_ap, idx_sb, tile0, ntiles,
                                       qn=0):
                            # 1024-idx chunks alternating queues qn / qn+2
                            done = 0
                            ci = 0
                            while done < ntiles:
                                k = min(8, ntiles - done)
                                nc.gpsimd.dma_gather(
                                    out_ap=gt[:, done:done + k, :],
                                    in_ap=src_ap,
                                    idxs_ap=idx_sb[:, (tile0 + done) * 8:
                                                   (tile0 + done + k) * 8],
                                    num_idxs=k * 128, num_idxs_reg=k * 128,
                                    elem_size=F_in, single_packet=False,
                                    queue_num=qn + 2 * (ci % 2))
                                done += k
                                ci += 1
                        if l == 0:
                            nc.sync.dma_start(
                                out=g_l[:],
                                in_=gx_in[:, (b0 * NT) * 128:
                                          (b0 * NT + span * TL) * 128]
                                    .rearrange("p (t f) -> p t f", f=F_in))
                            nc.sync.dma_start(
                                out=g_h[:],
                                in_=gx_in[:, (b0 * NT + span * TL) * 128:
                                          (b0 + span) * NT * 128]
                                    .rearrange("p (t f) -> p t f", f=F_in))
                        else:
                            gather_one(g_l, src_pool[:], gl_sb, b0 * TL,
                                       span * TL, qn=0)
                            gather_one(g_h, src_pool[BASE_B:, :], gh_sb,
                                       b0 * TH, span * TH, qn=1)
                        if span == 2 * GRP:
                            pair_cache[gi + 1] = (g_l, g_h, GRP)
                    m_sb = mpoolp.tile([128, nb * NT * 128], gdt,
                                       tag="m", bufs=2)
                    nc.sync.dma_start(
                        out=m_sb[:],
                        in_=mt_in[:, b0 * NT * 128:(b0 + nb) * NT * 128])

                    # aggregation into PSUM (per feature chunk)
                    agg_ps = [psA.tile([128, 512], dt.float32, tag="aggps",
                                       name=f"aggps{fc}", bufs=4)
                              for fc in range(FC)]
                    for bi, b in enumerate(blks):
                        for fc in range(FC):
                            o = agg_ps[fc][:, bi * 128:(bi + 1) * 128]
                            k = 0
                            for t in range(TL):
                                nc.tensor.matmul(
                                    out=o,
                                    lhsT=g_l[:, (toff + bi) * TL + t,
                                             fc * 128:(fc + 1) * 128],
                                    rhs=m_sb[:, (bi * NT + t) * 128:
                                             (bi * NT + t + 1) * 128],
                                    start=(k == 0), stop=False)
                                k += 1
                            for t in range(TH):
                                nc.tensor.matmul(
                                    out=o,
                                    lhsT=g_h[:, (toff + bi) * TH + t,
                                             fc * 128:(fc + 1) * 128],
                                    rhs=m_sb[:, (bi * NT + TL + t) * 128:
                                             (bi * NT + TL + t + 1) * 128],
                                    start=False, stop=False)
                            # self loop
                            nc.tensor.matmul(
                                out=o,
                                lhsT=u_src[:, b * F_in + fc * 128:
                                           b * F_in + (fc + 1) * 128],
                                rhs=ident[:], start=False, stop=True)

                    # BN_in correction + evac to SBUF bf16
                    agg_sb = [aggp.tile([128, 512], dt.bfloat16, tag="agg",
                                        name=f"aggsb{fc}", bufs=8)
                              for fc in range(FC)]
                    dslice = deg_sb[:, b0 * 128:(b0 + nb) * 128]  # [128, w]
                    for fc in range(FC):
                        w = nb * 128
                        nc.vector.tensor_scalar(
                            out=agg_sb[fc][:, :w], in0=agg_ps[fc][:, :w],
                            scalar1=pvec[:, PV(l, "gin", fc):
                                         PV(l, "gin", fc) + 1],
                            scalar2=None, op0=mybir.AluOpType.mult)
                        nc.vector.scalar_tensor_tensor(
                            out=agg_sb[fc][:, :w],
                            in0=dslice[:, :w],
                            scalar=pvec[:, PV(l, "bin", fc):
                                        PV(l, "bin", fc) + 1],
                            in1=agg_sb[fc][:, :w],
                            op0=mybir.AluOpType.mult,
                            op1=mybir.AluOpType.add)

                    # dense1: h1_T[m] = relu(sum_fc WA[fc,m].T @ agg[fc] + bA)
                    h1_sb = [h1p.tile([128, 512], dt.bfloat16, tag="h1",
                                      name=f"h1sb{m}", bufs=8)
                             for m in range(F1C)]
                    w = nb * 128
                    for m in range(F1C):
                        h1_ps = psB.tile([128, 512], dt.float32, tag="h1ps")
                        for fc in range(FC):
                            nc.tensor.matmul(
                                out=h1_ps[:, :w],
                                lhsT=wa_t[:, fc * f1w + m * 128:
                                          fc * f1w + (m + 1) * 128],
                                rhs=agg_sb[fc][:, :w],
                                start=(fc == 0), stop=(fc == FC - 1))
                        nc.scalar.activation(
                            out=h1_sb[m][:, :w], in_=h1_ps[:, :w],
                            func=mybir.ActivationFunctionType.Relu,
                            bias=pvec[:, PV(l, "bA", m):PV(l, "bA", m) + 1])

                    # dense2 per block: h2 = relu(sum_k h1[k,blk].T @ WB[k] + b)
                    u8_sb = (h1p.tile([128, GRP * HID], dt.float8e4,
                                      tag="u8", bufs=2, name="u8_sb")
                             if dst_bounce is not None else None)
                    for bi, b in enumerate(blks):
                        h2_ps = psC.tile([128, 512], dt.float32, tag="h2ps")
                        for k in range(F1C):
                            nc.tensor.matmul(
                                out=h2_ps[:],
                                lhsT=h1_sb[k][:, bi * 128:(bi + 1) * 128],
                                rhs=wb_t[:, k * 512:(k + 1) * 512],
                                start=(k == 0), stop=(k == F1C - 1))
                        src8 = None
                        if l == 0:
                            # bias carried by the constant-1 free row of h1
                            nc.scalar.activation(
                                out=u_loc[:, b * HID:(b + 1) * HID],
                                in_=h2_ps[:],
                                func=mybir.ActivationFunctionType.Relu)
                            src8 = h2_ps
                        else:
                            h2t = h1p.tile([128, 512], dt.bfloat16,
                                           tag="h2t", bufs=4, name="h2t")
                            nc.vector.tensor_tensor(
                                out=h2t[:], in0=h2_ps[:],
                                in1=browb[:, (l - 1) * 512:l * 512],
                                op=mybir.AluOpType.add)
                            nc.scalar.activation(
                                out=u_loc[:, b * HID:(b + 1) * HID],
                                in_=h2t[:],
                                func=mybir.ActivationFunctionType.Relu)
                            src8 = h2t
                        if dst_bounce is not None:
                            nc.scalar.activation(
                                out=u8_sb[:, bi * HID:(bi + 1) * HID],
                                in_=src8[:],
                                func=mybir.ActivationFunctionType.Relu)
                    if dst_bounce is not None:
                        nc.sync.dma_start(
                            out=dst_bounce[b0 * 128:(b0 + nb) * 128, :]
                                .rearrange("(b p) f -> p b f", p=128),
                            in_=u8_sb[:, :nb * HID]
                                .rearrange("p (b f) -> p b f", b=nb))
                if dst_pool is not None:
                    nc.gpsimd.collective_compute(
                        "AllGather", mybir.AluOpType.bypass,
                        replica_groups=[list(range(NC_))],
                        ins=[dst_bounce[:]], outs=[dst_pool[:]])

            # layer 0 (input conv): src pool_x, features 128
            with nc.named_scope("layer0"):
                conv_layer(0, None, N_FEAT, ux, pools_u[0], bounces_u[0])
            # conv layers 1..4
            for l in range(1, 5):
                sp = pools_u[l - 1]
                dp = pools_u[l] if l < 4 else None
                db = bounces_u[l] if l < 4 else None
                with nc.named_scope(f"layer{l}"):
                    conv_layer(l, sp, HID, u_loc, dp, db)
            conv_ctx.close()

            # ---------------- pooling into per-core graph window
            with tc.tile_pool(name="pps", bufs=4, space="PSUM") as pps, \
                 tc.tile_pool(name="mpp", bufs=2) as mpp, \
                 tc.tile_pool(name="winp", bufs=1) as winp:
                pool_ps = [pps.tile([128, PG], dt.float32, name=f"poolps{fc}",
                                    tag="poolps", bufs=4)
                           for fc in range(4)]
                for b in range(BLKS):
                    mp_sb = mpp.tile([128, PG], dt.bfloat16, tag="mp")
                    nc.sync.dma_start(out=mp_sb[:],
                                      in_=mp_in[:, b * PG:(b + 1) * PG])
                    for fc in range(4):
                        nc.tensor.matmul(
                            out=pool_ps[fc][:],
                            lhsT=u_loc[:, b * HID + fc * 128:
                                       b * HID + (fc + 1) * 128],
                            rhs=mp_sb[:],
                            start=(b == 0), stop=(b == BLKS - 1))
                win_sb = winp.tile([128, 4 * PG], dt.bfloat16)
                for fc in range(4):
                    nc.vector.tensor_copy(
                        out=win_sb[:, fc * PG:(fc + 1) * PG],
                        in_=pool_ps[fc][:])
                nc.sync.dma_start(
                    out=win_bounce[:].rearrange("(c p) g -> p c g", p=128),
                    in_=win_sb[:].rearrange("p (c g) -> p c g", c=4))
            nc.gpsimd.collective_compute(
                "AllGather", mybir.AluOpType.bypass,
                replica_groups=[list(range(NC_))],
                ins=[win_bounce[:]], outs=[wins_all[:]])

            # ---------------- reconstruction + head (redundant on all cores)
            with tc.tile_pool(name="headp", bufs=1) as hp, \
                 tc.tile_pool(name="wtmpp", bufs=4) as wtp, \
                 tc.tile_pool(name="hps", bufs=4, space="PSUM") as hps:
                pool_full = hp.tile([128, 4 * N_GRAPHS], dt.bfloat16)
                nc.vector.memset(pool_full[:], 0)
                return_wb = None  # placeholder
                # window bases are graph constants (same for all cores)
                for w in range(NC_):
                    wtmp = wtp.tile([128, 4 * PG], dt.bfloat16, tag="wtmp")
                    nc.sync.dma_start(
                        out=wtmp[:].rearrange("p (c g) -> p c g", c=4),
                        in_=wins_all[w * 512:(w + 1) * 512, :]
                            .rearrange("(c p) g -> p c g", p=128))
                    for fc in range(4):
                        dstv = pool_full[:, fc * N_GRAPHS + WBASES[w]:
                                         fc * N_GRAPHS + WBASES[w] + PG]
                        nc.vector.tensor_add(
                            out=dstv, in0=dstv,
                            in1=wtmp[:, fc * PG:(fc + 1) * PG])

                cur = pool_full
                for li in range(3):
                    nxt = hp.tile([128, 4 * N_GRAPHS], dt.bfloat16,
                                  name=f"head{li}", tag="headbuf", bufs=2)
                    for nk in range(4):
                        ns = slice(nk * 512, (nk + 1) * 512)
                        for m in range(4):
                            ps = hps.tile([128, 512], dt.float32, tag="hps")
                            for k in range(4):
                                nc.tensor.matmul(
                                    out=ps[:],
                                    lhsT=lw_sb[li][:, k * 512 + m * 128:
                                                   k * 512 + (m + 1) * 128],
                                    rhs=cur[:, k * N_GRAPHS + nk * 512:
                                            k * N_GRAPHS + (nk + 1) * 512],
                                    start=(k == 0), stop=(k == 3))
                            nc.scalar.activation(
                                out=nxt[:, m * N_GRAPHS + nk * 512:
                                        m * N_GRAPHS + (nk + 1) * 512],
                                in_=ps[:],
                                func=mybir.ActivationFunctionType.Relu,
                                bias=pvec[:, PV_LB(li, m):PV_LB(li, m) + 1])
                    cur = nxt
                osb = hp.tile([1, N_GRAPHS], dt.float32)
                for nk in range(4):
                    ps = hps.tile([1, 512], dt.float32, tag="ops")
                    for k in range(4):
                        nc.tensor.matmul(
                            out=ps[:],
                            lhsT=fw_sb[:, k:k + 1],
                            rhs=cur[:, k * N_GRAPHS + nk * 512:
                                    k * N_GRAPHS + (nk + 1) * 512],
                            start=(k == 0), stop=(k == 3))
                    nc.scalar.activation(
                        out=osb[:, nk * 512:(nk + 1) * 512], in_=ps[:],
                        func=mybir.ActivationFunctionType.Copy, bias=FB_CONST)
                nc.sync.dma_start(
                    out=out_ext[:].rearrange("g one -> one g"),
                    in_=osb[:])
    nc.compile()
    return nc


# WBASES / FB_CONST are module-level so build_device can see them; set in kernel()
WBASES = None
FB_CONST = 0.0


# ---------------------------------------------------------------- host packing

def make_in_maps(inputs, plan, layers):
    TL, TH = plan["TL"], plan["TH"]
    NT = TL + TH
    slot_of, core_of = plan["slot_of"], plan["core_of"]
    x = np.asarray(inputs["x"], np.float32)

    def wrap_idx(flat):
        """[N] int16 gather positions -> [128, N/16] wrapped+replicated."""
        n = len(flat)
        arr = flat.reshape(n // 16, 16).T.astype(np.int16)  # [16, n/16]
        return np.tile(arr, (8, 1))

    xpool_full = np.zeros((POOL, N_FEAT), np.float32)
    xpool_full[plan["pool_row"]] = x
    xpool_full = xpool_full.astype(BF16).astype(np.float32)

    in_maps = []
    for c in range(NC_):
        m = {}
        xs = np.zeros((SLOTS, N_FEAT), np.float32)
        nodes = np.arange(c * SHARD, (c + 1) * SHARD)
        xs[slot_of[nodes]] = x[nodes]
        m["x"] = xs.astype(BF16)

        # gather idx streams: per block, TL (or TH) tiles of 128
        gl = plan["idx"][c, :, :TL, :].reshape(-1)      # [BLKS*TL*128]
        gh = plan["idx"][c, :, TL:, :].reshape(-1)
        m["gl"] = wrap_idx(gl)
        m["gh"] = wrap_idx(gh)

        # layer-0 pre-gathered G in gather-output layout:
        # gx[p, (b*NT+t)*128 + f] = xpool[row(b,t,p)(+BASE_B for B), f]
        rows = plan["idx"][c].astype(np.int64).copy()    # [BLKS, NT, 128]
        rows[:, TL:, :] += BASE_B
        G0 = xpool_full[rows]                            # [BLKS, NT, 128, F]
        m["gx"] = np.ascontiguousarray(
            G0.transpose(2, 0, 1, 3).reshape(128, -1)).astype(BF16)

        # M tiles pre-swizzled: [128 e, BLKS*NT*128]
        mt = plan["M"][c].reshape(BLKS * NT, 128, 128)   # [tile, e, d]
        msw = np.ascontiguousarray(mt.transpose(1, 0, 2).reshape(128, -1))
        m["m"] = msw.astype(BF16)
        m["m8"] = msw.astype(ml_dtypes.float8_e4m3)

        mp = plan["mpool"][c]                            # [BLKS, 128, PG]
        m["mp"] = np.ascontiguousarray(
            mp.transpose(1, 0, 2).reshape(128, -1)).astype(BF16)

        m["deg"] = np.tile(plan["deg"][c][None, :],
                         (128, 1)).astype(BF16)

        pvec = np.zeros((128, 80), np.float32)
        for l in range(5):
            L = layers[l]
            FC = (N_FEAT if l == 0 else HID) // 128
            for fc in range(FC):
                pvec[:, 12 * l + 0 + fc] = L["gin"][fc * 128:(fc + 1) * 128]
                pvec[:, 12 * l + 4 + fc] = L["bin"][fc * 128:(fc + 1) * 128]
            F1C = 3 if l == 0 else 4
            ba = L["bA"]
            for mm in range(F1C):
                seg = ba[mm * 128:(mm + 1) * 128]
                pvec[:len(seg), 12 * l + 8 + mm] = seg
            if l == 0:
                pvec[64, 12 * l + 8 + 2] = 1.0  # free-row bias carrier
        for li in range(3):
            lb = np.asarray(inputs["lb"][li], np.float32)
            for mm in range(4):
                pvec[:, 60 + 4 * li + mm] = lb[mm * 128:(mm + 1) * 128]
        m["pvec"] = pvec

        brow = np.zeros((1, 5 * 512), np.float32)
        for l in range(5):
            brow[0, l * 512:(l + 1) * 512] = layers[l]["bB"]
        m["brow"] = brow.astype(BF16)
        browb = np.zeros((128, 4 * 512), np.float32)
        for l in range(1, 5):
            browb[:, (l - 1) * 512:l * 512] = layers[l]["bB"][None, :]
        m["browb"] = browb.astype(BF16)

        m["ident"] = np.eye(128, dtype=np.float32).astype(BF16)
        m["ones"] = np.ones((1, 128), np.float32).astype(BF16)

        wa0 = np.zeros((128, F1P), np.float32)
        wa0[:, :HID1] = layers[0]["WA"]                  # [128, 320]
        m["wa0"] = wa0.astype(BF16)
        wb0 = np.zeros((128, 3 * 512), np.float32)
        WB0 = layers[0]["WB"]                            # [320, 512]
        for k in range(3):
            seg = WB0[k * 128:(k + 1) * 128]
            wb0[:seg.shape[0], k * 512:(k + 1) * 512] = seg
        m["wb0"] = wb0.astype(BF16)

        wa = np.zeros((4, 128, 4 * 512), np.float32)
        wb = np.zeros((4, 128, 4 * 512), np.float32)
        for l in range(1, 5):
            WA, WBm = layers[l]["WA"], layers[l]["WB"]
            for fc in range(4):
                wa[l - 1, :, fc * 512:(fc + 1) * 512] = \
                    WA[fc * 128:(fc + 1) * 128, :]
                wb[l - 1, :, fc * 512:(fc + 1) * 512] = \
                    WBm[fc * 128:(fc + 1) * 128, :]
        m["wa"] = wa.astype(BF16)
        m["wb"] = wb.astype(BF16)

        lw = np.zeros((3, 128, 4 * 512), np.float32)
        for li in range(3):
            LW = np.asarray(inputs["lw"][li], np.float32)
            for k in range(4):
                for mm in range(4):
                    lw[li, :, k * 512 + mm * 128:k * 512 + (mm + 1) * 128] = \
                        LW[k * 128:(k + 1) * 128, mm * 128:(mm + 1) * 128]
        m["lw"] = lw.astype(BF16)

        fw = np.zeros((128, 4), np.float32)
        FW = np.asarray(inputs["fw"], np.float32)
        for k in range(4):
            fw[:, k] = FW[k * 128:(k + 1) * 128, 0]
        m["fw"] = fw.astype(BF16)

        in_maps.append(m)
    return in_maps


_CACHE = {}

def kernel(**inputs):
    global WBASES, FB_CONST
    from concourse.bass_utils import run_bass_kernel_spmd

    plan = build_plan(np.asarray(inputs["edge_index"]),
                      np.asarray(inputs["batch"]))
    layers = fold_params({k: np.asarray(v) for k, v in inputs.items()
                          if k not in ("x", "edge_index", "batch")})
    WBASES = [int(v) for v in plan["wbase"]]
    FB_CONST = float(np.asarray(inputs["fb"]).reshape(-1)[0])

    key = (plan["TL"], plan["TH"], tuple(WBASES), FB_CONST)
    if key not in _CACHE:
        _CACHE[key] = build_device(plan["TL"], plan["TH"])
    nc = _CACHE[key]

    in_maps = make_in_maps(inputs, plan, layers)
    res = run_bass_kernel_spmd(nc, in_maps, core_ids=list(range(NC_)),
                               trace=False)
    out = res.results[0]["out"].astype(np.float32)
    return out



# revision 12
# speedup vs baseline: 1.0245x; 1.0245x over previous
"""Trainium2 Bass kernel for nn_AqSolModel (GNN message passing), 8 NeuronCores.

Strategy (v1):
- Node-sharded: core c owns 6250 nodes, permuted into 49 blocks x 128 slots.
  Blocks 0-23 form chunk A (pool_a), blocks 24-48 chunk B (pool_b).
- Per layer the activation AllGather is split in two: AG-A (blocks 0-23)
  fires mid-layer and is hidden behind compute; only AG-B (~blocks 24-48)
  is exposed at the layer boundary. Gather stream A fetches sources living
  in chunk A (dep: AG-A only), stream B fetches chunk-B sources.
- Per-edge source rows fetched by dma_gather (int16 indices, one index
  space per chunk pool -- no base-offset tricks needed since each pool
  has < 32768 rows); segment-sum via matmuls against host-built 0/1
  selection tiles M (PSUM-accumulated per dst block) + identity matmul
  for the self loop.
- BatchNorms folded on host: BN_in's gain folded into W1; its bias term
  (bin*deg) and the dense1 bias enter as a K=2 rank-1 matmul
  (lhsT=[c_chunk; bA_chunk], rhs=[deg_row; ones_row]). BN_out folded into
  second dense weights/bias; dense2 bias enters as a K=1 rank-1 matmul.
- Activations stored fp8e4 everywhere off-chip; u_loc kept fp8 in SBUF and
  reused for the self loop, the bounce DMA and the pooling matmul (pooling
  matrices are exact 0/1; the 1/cnt scaling is applied after window
  reconstruction with a host-provided replicated row).
- Dense layers alternate matmul orientation so no transposes are needed.
- Mean-pool via per-block selection matmul into a per-core graph window;
  windows AllGathered and reconstructed on every core; small dense head
  runs redundantly on all cores; core 0's output is returned.

All index/selection data is computed on the host from edge_index/batch at
build time (the Bass graph is compiled after seeing the inputs), but all
feature compute runs on device.
"""
import sys
sys.path.insert(0, "/opt/trn_rl_repo")

import numpy as np
import ml_dtypes

BF16 = ml_dtypes.bfloat16
F8 = ml_dtypes.float8_e4m3

N_NODES, N_EDGES, N_FEAT, HID, HID1, N_GRAPHS, N_CONV, N_LIN = (
    50000, 150000, 128, 512, 320, 2048, 4, 3)
EPS = 1e-5
NC_ = 8
SHARD = N_NODES // NC_          # 6250
BLKS = 49
SLOTS = BLKS * 128              # 6272
CAB = 24                        # blocks in chunk A (groups 0-5)
CBB = BLKS - CAB                # 25 blocks in chunk B (groups 6-12)
CAS = CAB * 128                 # 3072 slots
CBS = CBB * 128                 # 3200 slots
PG = 384                        # pooling window width (3*128)
GRP = 4                         # blocks per gather/dense group
F1P = 384                       # HID1 padded to 3*128
AG_A_EMIT = 8                   # emit AG-A trigger after this group's gathers

# ---------------------------------------------------------------- host planning


def _pack2(degA, degB, nblk, capA, capB):
    """FFD-pack len(degA) nodes into nblk blocks of <=128 nodes s.t. per
    block sum(degA) <= capA and sum(degB) <= capB. Returns slot index
    (block*128+pos) or None."""
    n = len(degA)
    order = np.argsort(-(degA + degB))
    blk_cnt = np.zeros(nblk, np.int32)
    bA = np.zeros(nblk, np.int64)
    bB = np.zeros(nblk, np.int64)
    assign = np.full(n, -1, np.int32)
    for node in order:
        a, b2 = degA[node], degB[node]
        ok = (blk_cnt < 128) & (bA + a <= capA) & (bB + b2 <= capB)
        if not ok.any():
            return None
        cand = np.nonzero(ok)[0]
        j = cand[np.argmin(bA[cand] + bB[cand])]
        assign[node] = j
        blk_cnt[j] += 1
        bA[j] += a
        bB[j] += b2
    slot = np.full(n, -1, np.int32)
    nxt = np.zeros(nblk, np.int32)
    for node in range(n):
        j = assign[node]
        slot[node] = j * 128 + nxt[j]
        nxt[j] += 1
    return slot


def build_plan(edge_index, batch):
    src = edge_index[0].astype(np.int64)
    dst = edge_index[1].astype(np.int64)
    core_of = np.minimum(np.arange(N_NODES) // SHARD, NC_ - 1)
    deg_tot = np.bincount(dst, minlength=N_NODES)

    # phase 0: pack by total degree to get provisional chunk labels
    TL, TH = 2, 2
    slot0 = np.zeros(N_NODES, np.int64)
    for c in range(NC_):
        nodes = np.arange(c * SHARD, (c + 1) * SHARD)
        t = TL + TH
        while True:
            s = _pack2(deg_tot[nodes], np.zeros(SHARD, np.int64), BLKS,
                       t * 128, 1 << 30)
            if s is not None:
                break
            t += 1
        slot0[nodes] = s
    in_a = slot0 < CAS   # chunk label per node (source side), frozen now

    # per-node degrees toward A/B-sourced edges
    degA_n = np.bincount(dst[in_a[src]], minlength=N_NODES)
    degB_n = np.bincount(dst[~in_a[src]], minlength=N_NODES)

    # phase 1: repack each chunk of each core separately with stream caps
    slot_of = np.zeros(N_NODES, np.int64)
    while True:
        ok = True
        for c in range(NC_):
            nodes = np.arange(c * SHARD, (c + 1) * SHARD)
            la = in_a[nodes]
            na, nb = nodes[la], nodes[~la]
            if len(na) > CAS or len(nb) > CBS:
                raise RuntimeError("chunk overflow %d %d" % (len(na), len(nb)))
            sa = _pack2(degA_n[na], degB_n[na], CAB, TL * 128, TH * 128)
            sb = _pack2(degA_n[nb], degB_n[nb], CBB, TL * 128, TH * 128)
            if sa is None or sb is None:
                ok = False
                break
            slot_of[na] = sa
            slot_of[nb] = CAS + sb
        if ok:
            break
        if TL <= TH:
            TL += 1
        else:
            TH += 1
    NT = TL + TH

    # pool rows (per-chunk index spaces)
    assert CAS * NC_ <= 32768 and CBS * NC_ <= 32768
    prow = np.where(slot_of < CAS,
                    core_of * CAS + slot_of,
                    core_of * CBS + (slot_of - CAS))

    dst_core = core_of[dst]
    dst_slot = slot_of[dst]
    dst_blk = dst_slot // 128
    dst_col = dst_slot % 128
    src_in_a = in_a[src]

    idx_all = np.zeros((NC_, BLKS, NT, 128), np.int16)
    m_all = np.zeros((NC_, BLKS, NT, 128, 128), np.float32)
    snode = np.full((NC_, BLKS, NT, 128), -1, np.int64)
    for c in range(NC_):
        sel = dst_core == c
        e_idx = np.nonzero(sel)[0]
        b_of = dst_blk[e_idx]
        order = np.argsort(b_of, kind="stable")
        e_idx = e_idx[order]
        b_of = b_of[order]
        bounds = np.searchsorted(b_of, np.arange(BLKS + 1))
        for b in range(BLKS):
            es = e_idx[bounds[b]:bounds[b + 1]]
            a_es = es[src_in_a[es]]
            b_es = es[~src_in_a[es]]
            assert len(a_es) <= TL * 128 and len(b_es) <= TH * 128, (c, b)
            for eset, t0 in ((a_es, 0), (b_es, TL)):
                rel = prow[src[eset]]
                t = t0 + np.arange(len(eset)) // 128
                r = np.arange(len(eset)) % 128
                idx_all[c, b, t, r] = rel.astype(np.int16)
                snode[c, b, t, r] = src[eset]
                m_all[c, b, t, r, dst_col[eset]] = 1.0

    deg = np.bincount(dst, minlength=N_NODES).astype(np.float32) + 1.0
    deg_slots = np.zeros((NC_, SLOTS), np.float32)
    deg_slots[core_of, slot_of] = deg

    # pooling
    cnt = np.bincount(batch, minlength=N_GRAPHS).astype(np.float32)
    inv_cnt = (1.0 / np.maximum(cnt, 1.0)).astype(np.float32)
    g_of = batch.astype(np.int64)
    wbase = np.zeros(NC_, np.int32)
    mpool = np.zeros((NC_, BLKS, 128, PG), np.float32)
    for c in range(NC_):
        nodes = np.arange(c * SHARD, (c + 1) * SHARD)
        gmin, gmax = g_of[nodes].min(), g_of[nodes].max()
        wb = min(max(0, (gmin + gmax + 1) // 2 - PG // 2), N_GRAPHS - PG)
        wb = min(wb, gmin)
        wb = max(wb, gmax - PG + 1)
        assert wb >= 0 and wb + PG <= N_GRAPHS and gmin >= wb and gmax < wb + PG, \
            (c, gmin, gmax, wb)
        wbase[c] = wb
        cols = slot_of[nodes] % 128
        blks = slot_of[nodes] // 128
        mpool[c, blks, cols, g_of[nodes] - wb] = 1.0

    return dict(slot_of=slot_of, core_of=core_of,
                TL=TL, TH=TH, idx=idx_all, M=m_all, deg=deg_slots,
                snode=snode, mpool=mpool, wbase=wbase, inv_cnt=inv_cnt)


def fold_params(p):
    out = []
    for l in range(5):
        if l == 0:
            ing, inb, inm, inv = p['in_g1'], p['in_b1'], p['in_m1'], p['in_v1']
            wa, ba, wb, bb = p['w1a'], p['b1a'], p['w1b'], p['b1b']
            og, ob, om, ov = p['out_g1'], p['out_b1'], p['out_m1'], p['out_v1']
        else:
            i = l - 1
            ing, inb, inm, inv = (p['cin_g'][i], p['cin_b'][i],
                                  p['cin_m'][i], p['cin_v'][i])
            wa, ba, wb, bb = p['cwA'][i], p['cbA'][i], p['cwB'][i], p['cbB'][i]
            og, ob, om, ov = (p['cout_g'][i], p['cout_b'][i],
                              p['cout_m'][i], p['cout_v'][i])
        gin = np.asarray(ing / np.sqrt(inv + EPS), np.float64)
        bin_ = np.asarray(inb - inm * gin, np.float64)
        gout = np.asarray(og / np.sqrt(ov + EPS), np.float64)
        bout = np.asarray(ob - om * gout, np.float64)
        WA = np.asarray(wa, np.float64) * gin[:, None]   # BN-in gain folded
        cvec = np.asarray(wa, np.float64).T @ bin_       # [HID1]: deg coeff
        WB = np.asarray(wb, np.float64) * gout[None, :]
        bB = np.asarray(bb, np.float64) * gout + bout
        out.append(dict(WA=np.asarray(WA, np.float32),
                        cvec=np.asarray(cvec, np.float32),
                        bA=np.asarray(ba, np.float32),
                        WB=np.asarray(WB, np.float32),
                        bB=np.asarray(bB, np.float32)))
    return out


# ---------------------------------------------------------------- device build


def build_device(TL, TH):
    """Build the Bacc graph (shapes only; all data arrives via in_maps)."""
    from concourse import bass, bacc, mybir, tile

    NT = TL + TH
    dt = mybir.dt
    nc = bacc.Bacc("TRN2", target_bir_lowering=False, debug=False,
                   enable_asserts=False, num_devices=NC_,
                   num_swdge_queues=4)

    def inp(name, shape, dtype):
        return nc.dram_tensor(name, shape, dtype, kind="ExternalInput")

    x_in = inp("x", [SLOTS, N_FEAT], dt.bfloat16)
    gx_in = inp("gx", [128, BLKS * NT * 128], dt.float8e4)
    gl_in = inp("gl", [128, BLKS * TL * 8], dt.int16)
    gh_in = inp("gh", [128, BLKS * TH * 8], dt.int16)
    m8_in = inp("m8", [128, BLKS * NT * 128], dt.float8e4)
    mp_in = inp("mp", [128, BLKS * PG], dt.float8e4)
    rkw_in = inp("rkw", [2, 5 * 512], dt.bfloat16)
    rkr_in = inp("rkr", [2, SLOTS], dt.bfloat16)
    invc_in = inp("invc", [128, N_GRAPHS], dt.bfloat16)
    pvec_in = inp("pvec", [128, 16], dt.float32)
    brow_in = inp("brow", [1, 5 * 512], dt.bfloat16)
    ones_in = inp("ones", [1, 128], dt.bfloat16)
    ident_in = inp("ident", [128, 128], dt.bfloat16)
    ident8_in = inp("ident8", [128, 128], dt.float8e4)
    wa0_in = inp("wa0", [128, F1P], dt.bfloat16)
    wb0_in = inp("wb0", [128, 3 * 512], dt.bfloat16)
    wa_in = inp("wa", [4, 128, 4 * 512], dt.bfloat16)
    wb_in = inp("wb", [4, 128, 4 * 512], dt.bfloat16)
    lw_in = inp("lw", [3, 128, 4 * 512], dt.bfloat16)
    fw_in = inp("fw", [128, 4], dt.bfloat16)
    out_ext = nc.dram_tensor("out", [N_GRAPHS, 1], dt.float32,
                             kind="ExternalOutput")

    # group structure: 12 groups of 4 blocks + 1 group of 1 block
    groups = [list(range(g * GRP, min((g + 1) * GRP, BLKS)))
              for g in range((BLKS + GRP - 1) // GRP)]

    PV_LB = lambda l, chunk: 4 * l + chunk

    with tile.TileContext(nc) as tc:
        import contextlib
        ctx = contextlib.ExitStack()
        with ctx:
            dram = ctx.enter_context(tc.tile_pool(name="dram", bufs=1,
                                                  space="DRAM"))
            const = ctx.enter_context(tc.tile_pool(name="const", bufs=1))

            # DRAM: per-layer chunked activation pools + bounces
            pool_a = [dram.tile([NC_ * CAS, HID], dt.float8e4,
                                addr_space="Shared", name=f"pool_a{i}")
                      for i in range(4)]
            pool_b = [dram.tile([NC_ * CBS, HID], dt.float8e4,
                                addr_space="Shared", name=f"pool_b{i}")
                      for i in range(4)]
            bounce_a = [dram.tile([CAS, HID], dt.float8e4,
                                  name=f"bounce_a{i}") for i in range(4)]
            bounce_b = [dram.tile([CBS, HID], dt.float8e4,
                                  name=f"bounce_b{i}") for i in range(4)]
            win_bounce = dram.tile([4 * 128, PG], dt.bfloat16)
            wins_all = dram.tile([NC_ * 4 * 128, PG], dt.bfloat16,
                                 addr_space="Shared")

            # persistent SBUF
            gl_sb = const.tile([128, BLKS * TL * 8], dt.int16)
            gh_sb = const.tile([128, BLKS * TH * 8], dt.int16)
            ux = const.tile([128, BLKS * N_FEAT], dt.bfloat16)
            u_loc = const.tile([128, BLKS * HID], dt.float8e4)
            rkw = const.tile([2, 5 * 512], dt.bfloat16)
            rkr = const.tile([2, SLOTS], dt.bfloat16)
            invc = const.tile([128, N_GRAPHS], dt.bfloat16)
            pvec = const.tile([128, 16], dt.float32)
            brow = const.tile([1, 5 * 512], dt.bfloat16)
            onesr = const.tile([1, 128], dt.bfloat16)
            ident = const.tile([128, 128], dt.bfloat16)
            ident8 = const.tile([128, 128], dt.float8e4)
            wa0 = const.tile([128, F1P], dt.bfloat16)
            wb0 = const.tile([128, 3 * 512], dt.bfloat16)
            wa_sb = [const.tile([128, 4 * 512], dt.bfloat16, name=f"wa{i}")
                     for i in range(4)]
            wb_sb = [const.tile([128, 4 * 512], dt.bfloat16, name=f"wb{i}")
                     for i in range(4)]
            lw_sb = [const.tile([128, 4 * 512], dt.bfloat16, name=f"lwt{i}")
                     for i in range(3)]
            fw_sb = const.tile([128, 4], dt.bfloat16)

            nc.sync.dma_start(out=gl_sb[:], in_=gl_in[:])
            nc.sync.dma_start(out=gh_sb[:], in_=gh_in[:])
            nc.sync.dma_start(out=rkw[:], in_=rkw_in[:])
            nc.sync.dma_start(out=rkr[:], in_=rkr_in[:])
            nc.sync.dma_start(out=invc[:], in_=invc_in[:])
            nc.sync.dma_start(out=pvec[:], in_=pvec_in[:])
            nc.sync.dma_start(out=brow[:], in_=brow_in[:])
            nc.sync.dma_start(out=onesr[:], in_=ones_in[:])
            nc.sync.dma_start(out=ident[:], in_=ident_in[:])
            nc.sync.dma_start(out=ident8[:], in_=ident8_in[:])
            nc.sync.dma_start(out=wa0[:], in_=wa0_in[:])
            nc.sync.dma_start(out=wb0[:], in_=wb0_in[:])
            for i in range(4):
                nc.sync.dma_start(out=wa_sb[i][:], in_=wa_in[i])
                nc.sync.dma_start(out=wb_sb[i][:], in_=wb_in[i])
            for i in range(3):
                nc.sync.dma_start(out=lw_sb[i][:], in_=lw_in[i])
            nc.sync.dma_start(out=fw_sb[:], in_=fw_in[:])

            nc.sync.dma_start(
                out=ux[:].rearrange("p (b f) -> p b f", b=BLKS),
                in_=x_in[:].rearrange("(b p) f -> p b f", p=128))

            conv_ctx = contextlib.ExitStack()
            gpool = conv_ctx.enter_context(tc.tile_pool(name="gpool", bufs=2))
            mpoolp = conv_ctx.enter_context(tc.tile_pool(name="mpoolp", bufs=2))
            aggp = conv_ctx.enter_context(tc.tile_pool(name="aggp", bufs=8))
            h1p = conv_ctx.enter_context(tc.tile_pool(name="h1p", bufs=8))
            psA = conv_ctx.enter_context(tc.tile_pool(name="psA", bufs=4,
                                                      space="PSUM"))
            psB = conv_ctx.enter_context(tc.tile_pool(name="psB", bufs=2,
                                                      space="PSUM"))
            psC = conv_ctx.enter_context(tc.tile_pool(name="psC", bufs=2,
                                                      space="PSUM"))

            def conv_layer(l, src_a, src_b, u_src, dst_a, dst_b, bnc_a, bnc_b):
                """One sumconv layer. u_src: fp8 (or bf16 for l=0) SBUF tile of
                local activations (selfloop source, [128, BLKS*F_in])."""
                FC = F_in = None
                F_in = N_FEAT if l == 0 else HID
                FC = F_in // 128
                F1C = 3 if l == 0 else 4
                wa_t = wa0 if l == 0 else wa_sb[l - 1]
                wb_t = wb0 if l == 0 else wb_sb[l - 1]
                f1w = F1P if l == 0 else 512
                id_t = ident if l == 0 else ident8
                u_out = u_loc
                pair_cache = {}
                for gi, blks in enumerate(groups):
                    nb = len(blks)
                    b0 = blks[0]
                    if gi in pair_cache:
                        g_l, g_h, toff = pair_cache.pop(gi)
                    else:
                        toff = 0
                        span = nb
                        if (nb == GRP and gi + 1 < len(groups)
                                and len(groups[gi + 1]) == GRP):
                            span = 2 * GRP
                        g_l = gpool.tile([128, span * TL, F_in], dt.float8e4,
                                         tag="gl", bufs=2)
                        g_h = gpool.tile([128, span * TH, F_in], dt.float8e4,
                                         tag="gh", bufs=2)

                        def gather_one(gt, src_ap, idx_sb, tile0, ntiles,
                                       qn=0):
                            done = 0
                            ci = 0
                            while done < ntiles:
                                k = min(8, ntiles - done)
                                nc.gpsimd.dma_gather(
                                    out_ap=gt[:, done:done + k, :],
                                    in_ap=src_ap,
                                    idxs_ap=idx_sb[:, (tile0 + done) * 8:
                                                   (tile0 + done + k) * 8],
                                    num_idxs=k * 128, num_idxs_reg=k * 128,
                                    elem_size=F_in, single_packet=False,
                                    queue_num=qn + 2 * (ci % 2))
                                done += k
                                ci += 1
                        if l == 0:
                            nc.sync.dma_start(
                                out=g_l[:],
                                in_=gx_in[:, (b0 * NT) * 128:
                                          (b0 * NT + span * TL) * 128]
                                    .rearrange("p (t f) -> p t f", f=F_in))
                            nc.sync.dma_start(
                                out=g_h[:],
                                in_=gx_in[:, (b0 * NT + span * TL) * 128:
                                          (b0 + span) * NT * 128]
                                    .rearrange("p (t f) -> p t f", f=F_in))
                        else:
                            gather_one(g_l, src_a[:], gl_sb, b0 * TL,
                                       span * TL, qn=0)
                            gather_one(g_h, src_b[:], gh_sb, b0 * TH,
                                       span * TH, qn=1)
                        if span == 2 * GRP:
                            pair_cache[gi + 1] = (g_l, g_h, GRP)

                    # AG-A trigger for THIS layer's output: emitted after
                    # group AG_A_EMIT's gathers so it doesn't stall the
                    # gather FIFO, fires once bounce_a (groups 0-5) lands.
                    if gi == AG_A_EMIT and dst_a is not None:
                        nc.gpsimd.collective_compute(
                            "AllGather", mybir.AluOpType.bypass,
                            replica_groups=[list(range(NC_))],
                            ins=[bnc_a[:]], outs=[dst_a[:]])

                    m_sb = mpoolp.tile([128, nb * NT * 128], dt.float8e4,
                                       tag="m", bufs=2)
                    nc.sync.dma_start(
                        out=m_sb[:],
                        in_=m8_in[:, b0 * NT * 128:(b0 + nb) * NT * 128])

                    # aggregation into PSUM (per feature chunk)
                    agg_ps = [psA.tile([128, 512], dt.float32, tag="aggps",
                                       name=f"aggps{fc}", bufs=4)
                              for fc in range(FC)]
                    for bi, b in enumerate(blks):
                        for fc in range(FC):
                            o = agg_ps[fc][:, bi * 128:(bi + 1) * 128]
                            k = 0
                            for t in range(TL):
                                nc.tensor.matmul(
                                    out=o,
                                    lhsT=g_l[:, (toff + bi) * TL + t,
                                             fc * 128:(fc + 1) * 128],
                                    rhs=m_sb[:, (bi * NT + t) * 128:
                                             (bi * NT + t + 1) * 128],
                                    start=(k == 0), stop=False)
                                k += 1
                            for t in range(TH):
                                nc.tensor.matmul(
                                    out=o,
                                    lhsT=g_h[:, (toff + bi) * TH + t,
                                             fc * 128:(fc + 1) * 128],
                                    rhs=m_sb[:, (bi * NT + TL + t) * 128:
                                             (bi * NT + TL + t + 1) * 128],
                                    start=False, stop=False)
                            # self loop (raw activations; BN handled via
                            # folded weights + rank-1 term)
                            nc.tensor.matmul(
                                out=o,
                                lhsT=u_src[:, b * F_in + fc * 128:
                                           b * F_in + (fc + 1) * 128],
                                rhs=id_t[:], start=False, stop=True)

                    # evac raw aggregate to SBUF bf16
                    agg_sb = [aggp.tile([128, 512], dt.bfloat16, tag="agg",
                                        name=f"aggsb{fc}", bufs=8)
                              for fc in range(FC)]
                    w = nb * 128
                    for fc in range(FC):
                        nc.vector.tensor_copy(
                            out=agg_sb[fc][:, :w], in_=agg_ps[fc][:, :w])

                    # dense1: h1_T[m] = relu(sum_fc WA'[fc,m].T @ agg[fc]
                    #                        + cvec[m] x deg + bA[m] x 1)
                    h1_sb = [h1p.tile([128, 512], dt.bfloat16, tag="h1",
                                      name=f"h1sb{m}", bufs=8)
                             for m in range(F1C)]
                    for m in range(F1C):
                        h1_ps = psB.tile([128, 512], dt.float32, tag="h1ps")
                        for fc in range(FC):
                            nc.tensor.matmul(
                                out=h1_ps[:, :w],
                                lhsT=wa_t[:, fc * f1w + m * 128:
                                          fc * f1w + (m + 1) * 128],
                                rhs=agg_sb[fc][:, :w],
                                start=(fc == 0), stop=False)
                        nc.tensor.matmul(
                            out=h1_ps[:, :w],
                            lhsT=rkw[:, l * 512 + m * 128:
                                     l * 512 + (m + 1) * 128],
                            rhs=rkr[:, b0 * 128:b0 * 128 + w],
                            start=False, stop=True)
                        nc.scalar.activation(
                            out=h1_sb[m][:, :w], in_=h1_ps[:, :w],
                            func=mybir.ActivationFunctionType.Relu)

                    # dense2 per block: u = relu(sum_k h1[k,blk].T @ WB[k]
                    #                            + ones x bB)
                    for bi, b in enumerate(blks):
                        h2_ps = psC.tile([128, 512], dt.float32, tag="h2ps")
                        for k in range(F1C):
                            nc.tensor.matmul(
                                out=h2_ps[:],
                                lhsT=h1_sb[k][:, bi * 128:(bi + 1) * 128],
                                rhs=wb_t[:, k * 512:(k + 1) * 512],
                                start=(k == 0), stop=False)
                        nc.tensor.matmul(
                            out=h2_ps[:],
                            lhsT=onesr[:],
                            rhs=brow[:, l * 512:(l + 1) * 512],
                            start=False, stop=True)
                        nc.scalar.activation(
                            out=u_out[:, b * HID:(b + 1) * HID],
                            in_=h2_ps[:],
                            func=mybir.ActivationFunctionType.Relu)
                    if bnc_a is not None:
                        if b0 < CAB:  # groups 0-5 -> chunk A bounce
                            nc.sync.dma_start(
                                out=bnc_a[b0 * 128:(b0 + nb) * 128, :]
                                    .rearrange("(b p) f -> p b f", p=128),
                                in_=u_out[:, b0 * HID:(b0 + nb) * HID]
                                    .rearrange("p (b f) -> p b f", b=nb))
                        else:
                            c0 = b0 - CAB
                            nc.sync.dma_start(
                                out=bnc_b[c0 * 128:(c0 + nb) * 128, :]
                                    .rearrange("(b p) f -> p b f", p=128),
                                in_=u_out[:, b0 * HID:(b0 + nb) * HID]
                                    .rearrange("p (b f) -> p b f", b=nb))
                if dst_b is not None:
                    nc.gpsimd.collective_compute(
                        "AllGather", mybir.AluOpType.bypass,
                        replica_groups=[list(range(NC_))],
                        ins=[bnc_b[:]], outs=[dst_b[:]])

            # layer 0 (input conv, gx pre-gathered): writes pools 0
            with nc.named_scope("layer0"):
                conv_layer(0, None, None, ux, pool_a[0], pool_b[0],
                           bounce_a[0], bounce_b[0])
            for l in range(1, 5):
                sa, sb2 = pool_a[l - 1], pool_b[l - 1]
                da = pool_a[l] if l < 4 else None
                db = pool_b[l] if l < 4 else None
                ba2 = bounce_a[l] if l < 4 else None
                bb2 = bounce_b[l] if l < 4 else None
                with nc.named_scope(f"layer{l}"):
                    conv_layer(l, sa, sb2, u_loc, da, db, ba2, bb2)
            conv_ctx.close()

            # ---------------- pooling into per-core graph window
            with tc.tile_pool(name="pps", bufs=4, space="PSUM") as pps, \
                 tc.tile_pool(name="mpp", bufs=2) as mpp, \
                 tc.tile_pool(name="winp", bufs=1) as winp:
                pool_ps = [pps.tile([128, PG], dt.float32, name=f"poolps{fc}",
                                    tag="poolps", bufs=4)
                           for fc in range(4)]
                for b in range(BLKS):
                    mp_sb = mpp.tile([128, PG], dt.float8e4, tag="mp")
                    nc.sync.dma_start(out=mp_sb[:],
                                      in_=mp_in[:, b * PG:(b + 1) * PG])
                    for fc in range(4):
                        nc.tensor.matmul(
                            out=pool_ps[fc][:],
                            lhsT=u_loc[:, b * HID + fc * 128:
                                       b * HID + (fc + 1) * 128],
                            rhs=mp_sb[:],
                            start=(b == 0), stop=(b == BLKS - 1))
                win_sb = winp.tile([128, 4 * PG], dt.bfloat16)
                for fc in range(4):
                    nc.vector.tensor_copy(
                        out=win_sb[:, fc * PG:(fc + 1) * PG],
                        in_=pool_ps[fc][:])
                nc.sync.dma_start(
                    out=win_bounce[:].rearrange("(c p) g -> p c g", p=128),
                    in_=win_sb[:].rearrange("p (c g) -> p c g", c=4))
            nc.gpsimd.collective_compute(
                "AllGather", mybir.AluOpType.bypass,
                replica_groups=[list(range(NC_))],
                ins=[win_bounce[:]], outs=[wins_all[:]])

            # ---------------- reconstruction + head (redundant on all cores)
            with tc.tile_pool(name="headp", bufs=1) as hp, \
                 tc.tile_pool(name="wtmpp", bufs=4) as wtp, \
                 tc.tile_pool(name="hps", bufs=4, space="PSUM") as hps:
                pool_full = hp.tile([128, 4 * N_GRAPHS], dt.bfloat16)
                nc.vector.memset(pool_full[:], 0)
                for w in range(NC_):
                    wtmp = wtp.tile([128, 4 * PG], dt.bfloat16, tag="wtmp")
                    nc.sync.dma_start(
                        out=wtmp[:].rearrange("p (c g) -> p c g", c=4),
                        in_=wins_all[w * 512:(w + 1) * 512, :]
                            .rearrange("(c p) g -> p c g", p=128))
                    for fc in range(4):
                        dstv = pool_full[:, fc * N_GRAPHS + WBASES[w]:
                                         fc * N_GRAPHS + WBASES[w] + PG]
                        nc.vector.tensor_add(
                            out=dstv, in0=dstv,
                            in1=wtmp[:, fc * PG:(fc + 1) * PG])
                # mean-pool normalization (sums -> means)
                for fc in range(4):
                    nc.vector.tensor_tensor(
                        out=pool_full[:, fc * N_GRAPHS:(fc + 1) * N_GRAPHS],
                        in0=pool_full[:, fc * N_GRAPHS:(fc + 1) * N_GRAPHS],
                        in1=invc[:],
                        op=mybir.AluOpType.mult)

                cur = pool_full
                for li in range(3):
                    nxt = hp.tile([128, 4 * N_GRAPHS], dt.bfloat16,
                                  name=f"head{li}", tag="headbuf", bufs=2)
                    for nk in range(4):
                        for m in range(4):
                            ps = hps.tile([128, 512], dt.float32, tag="hps")
                            for k in range(4):
                                nc.tensor.matmul(
                                    out=ps[:],
                                    lhsT=lw_sb[li][:, k * 512 + m * 128:
                                                   k * 512 + (m + 1) * 128],
                                    rhs=cur[:, k * N_GRAPHS + nk * 512:
                                            k * N_GRAPHS + (nk + 1) * 512],
                                    start=(k == 0), stop=(k == 3))
                            nc.scalar.activation(
                                out=nxt[:, m * N_GRAPHS + nk * 512:
                                        m * N_GRAPHS + (nk + 1) * 512],
                                in_=ps[:],
                                func=mybir.ActivationFunctionType.Relu,
                                bias=pvec[:, PV_LB(li, m):PV_LB(li, m) + 1])
                    cur = nxt
                osb = hp.tile([1, N_GRAPHS], dt.float32)
                for nk in range(4):
                    ps = hps.tile([1, 512], dt.float32, tag="ops")
                    for k in range(4):
                        nc.tensor.matmul(
                            out=ps[:],
                            lhsT=fw_sb[:, k:k + 1],
                            rhs=cur[:, k * N_GRAPHS + nk * 512:
                                    k * N_GRAPHS + (nk + 1) * 512],
                            start=(k == 0), stop=(k == 3))
                    nc.scalar.activation(
                        out=osb[:, nk * 512:(nk + 1) * 512], in_=ps[:],
                        func=mybir.ActivationFunctionType.Copy, bias=FB_CONST)
                nc.sync.dma_start(
                    out=out_ext[:].rearrange("g one -> one g"),
                    in_=osb[:])
    nc.compile()
    return nc


# WBASES / FB_CONST are module-level so build_device can see them; set in kernel()
WBASES = None
FB_CONST = 0.0


# ---------------------------------------------------------------- host packing


def make_in_maps(inputs, plan, layers):
    TL, TH = plan["TL"], plan["TH"]
    NT = TL + TH
    slot_of, core_of = plan["slot_of"], plan["core_of"]
    x = np.asarray(inputs["x"], np.float32)
    x8 = x.astype(F8).astype(np.float32)

    def wrap_idx(flat):
        """[N] int16 gather positions -> [128, N/16] wrapped+replicated."""
        n = len(flat)
        arr = flat.reshape(n // 16, 16).T.astype(np.int16)  # [16, n/16]
        return np.tile(arr, (8, 1))

    in_maps = []
    for c in range(NC_):
        m = {}
        xs = np.zeros((SLOTS, N_FEAT), np.float32)
        nodes = np.arange(c * SHARD, (c + 1) * SHARD)
        xs[slot_of[nodes]] = x[nodes]
        m["x"] = xs.astype(BF16)

        gl = plan["idx"][c, :, :TL, :].reshape(-1)
        gh = plan["idx"][c, :, TL:, :].reshape(-1)
        m["gl"] = wrap_idx(gl)
        m["gh"] = wrap_idx(gh)

        # layer-0 pre-gathered G, span-grouped to match device consumption:
        # per span of blocks: A-tiles (block-major, t<TL) then B-tiles.
        sn = plan["snode"][c]                            # [BLKS, NT, 128]
        G0 = np.where(sn[..., None] >= 0,
                      x8[np.maximum(sn, 0)], 0.0)        # [BLKS, NT, 128, F]
        gx = np.zeros((128, BLKS * NT * 128), np.float32)
        col = 0
        b0s = 0
        while b0s < BLKS:
            span = min(2 * GRP, BLKS - b0s)
            if span != 2 * GRP:
                span = BLKS - b0s if BLKS - b0s < GRP else GRP
            for b in range(b0s, b0s + span):
                for t in range(TL):
                    gx[:, col:col + N_FEAT] = G0[b, t]
                    col += N_FEAT
            for b in range(b0s, b0s + span):
                for t in range(TL, NT):
                    gx[:, col:col + N_FEAT] = G0[b, t]
                    col += N_FEAT
            b0s += span
        m["gx"] = gx.astype(F8)

        mt = plan["M"][c].reshape(BLKS * NT, 128, 128)
        msw = np.ascontiguousarray(mt.transpose(1, 0, 2).reshape(128, -1))
        m["m8"] = msw.astype(F8)

        mp = plan["mpool"][c]                            # [BLKS, 128, PG]
        m["mp"] = np.ascontiguousarray(
            mp.transpose(1, 0, 2).reshape(128, -1)).astype(F8)

        rkw = np.zeros((2, 5 * 512), np.float32)
        for l in range(5):
            L = layers[l]
            n1 = len(L["cvec"])                          # 320 or 512
            rkw[0, l * 512:l * 512 + n1] = L["cvec"]
            rkw[1, l * 512:l * 512 + n1] = L["bA"]
        m["rkw"] = rkw.astype(BF16)

        rkr = np.zeros((2, SLOTS), np.float32)
        rkr[0] = plan["deg"][c]
        rkr[1] = 1.0
        m["rkr"] = rkr.astype(BF16)

        m["invc"] = np.tile(plan["inv_cnt"][None, :], (128, 1)).astype(BF16)

        pvec = np.zeros((128, 16), np.float32)
        for li in range(3):
            lb = np.asarray(inputs["lb"][li], np.float32)
            for mm in range(4):
                pvec[:, 4 * li + mm] = lb[mm * 128:(mm + 1) * 128]
        m["pvec"] = pvec

        brow = np.zeros((1, 5 * 512), np.float32)
        for l in range(5):
            brow[0, l * 512:(l + 1) * 512] = layers[l]["bB"]
        m["brow"] = brow.astype(BF16)

        m["ones"] = np.ones((1, 128), np.float32).astype(BF16)
        m["ident"] = np.eye(128, dtype=np.float32).astype(BF16)
        m["ident8"] = np.eye(128, dtype=np.float32).astype(F8)

        wa0 = np.zeros((128, F1P), np.float32)
        wa0[:, :HID1] = layers[0]["WA"]
        m["wa0"] = wa0.astype(BF16)
        wb0 = np.zeros((128, 3 * 512), np.float32)
        WB0 = layers[0]["WB"]
        for k in range(3):
            seg = WB0[k * 128:(k + 1) * 128]
            wb0[:seg.shape[0], k * 512:(k + 1) * 512] = seg
        m["wb0"] = wb0.astype(BF16)

        wa = np.zeros((4, 128, 4 * 512), np.float32)
        wb = np.zeros((4, 128, 4 * 512), np.float32)
        for l in range(1, 5):
            WA, WBm = layers[l]["WA"], layers[l]["WB"]
            for fc in range(4):
                wa[l - 1, :, fc * 512:(fc + 1) * 512] = \
                    WA[fc * 128:(fc + 1) * 128, :]
                wb[l - 1, :, fc * 512:(fc + 1) * 512] = \
                    WBm[fc * 128:(fc + 1) * 128, :]
        m["wa"] = wa.astype(BF16)
        m["wb"] = wb.astype(BF16)

        lw = np.zeros((3, 128, 4 * 512), np.float32)
        for li in range(3):
            LW = np.asarray(inputs["lw"][li], np.float32)
            for k in range(4):
                for mm in range(4):
                    lw[li, :, k * 512 + mm * 128:k * 512 + (mm + 1) * 128] = \
                        LW[k * 128:(k + 1) * 128, mm * 128:(mm + 1) * 128]
        m["lw"] = lw.astype(BF16)

        fw = np.zeros((128, 4), np.float32)
        FW = np.asarray(inputs["fw"], np.float32)
        for k in range(4):
            fw[:, k] = FW[k * 128:(k + 1) * 128, 0]
        m["fw"] = fw.astype(BF16)

        in_maps.append(m)
    return in_maps


_CACHE = {}


def kernel(**inputs):
    global WBASES, FB_CONST
    from concourse.bass_utils import run_bass_kernel_spmd

    plan = build_plan(np.asarray(inputs["edge_index"]),
                      np.asarray(inputs["batch"]))
    layers = fold_params({k: np.asarray(v) for k, v in inputs.items()
                          if k not in ("x", "edge_index", "batch")})
    WBASES = [int(v) for v in plan["wbase"]]
    FB_CONST = float(np.asarray(inputs["fb"]).reshape(-1)[0])

    key = (plan["TL"], plan["TH"], tuple(WBASES), FB_CONST)
    if key not in _CACHE:
        _CACHE[key] = build_device(plan["TL"], plan["TH"])
    nc = _CACHE[key]

    in_maps = make_in_maps(inputs, plan, layers)
    res = run_bass_kernel_spmd(nc, in_maps, core_ids=list(range(NC_)),
                               trace=False)
    out = res.results[0]["out"].astype(np.float32)
    return out


# revision 14
# speedup vs baseline: 1.0527x; 1.0275x over previous
"""Trainium2 Bass kernel for nn_AqSolModel (GNN message passing), 8 NeuronCores.

Strategy (v1):
- Node-sharded: core c owns 6250 nodes, permuted into 49 blocks x 128 slots.
  Blocks 0-23 form chunk A (pool_a), blocks 24-48 chunk B (pool_b).
- Per layer the activation AllGather is split in two: AG-A (blocks 0-23)
  fires mid-layer and is hidden behind compute; only AG-B (~blocks 24-48)
  is exposed at the layer boundary. Gather stream A fetches sources living
  in chunk A (dep: AG-A only), stream B fetches chunk-B sources.
- Per-edge source rows fetched by dma_gather (int16 indices, one index
  space per chunk pool -- no base-offset tricks needed since each pool
  has < 32768 rows); segment-sum via matmuls against host-built 0/1
  selection tiles M (PSUM-accumulated per dst block) + identity matmul
  for the self loop.
- BatchNorms folded on host: BN_in's gain folded into W1; its bias term
  (bin*deg) and the dense1 bias enter as a K=2 rank-1 matmul
  (lhsT=[c_chunk; bA_chunk], rhs=[deg_row; ones_row]). BN_out folded into
  second dense weights/bias; dense2 bias enters as a K=1 rank-1 matmul.
- Activations stored fp8e4 everywhere off-chip; u_loc kept fp8 in SBUF and
  reused for the self loop, the bounce DMA and the pooling matmul (pooling
  matrices are exact 0/1; the 1/cnt scaling is applied after window
  reconstruction with a host-provided replicated row).
- Dense layers alternate matmul orientation so no transposes are needed.
- Mean-pool via per-block selection matmul into a per-core graph window;
  windows AllGathered and reconstructed on every core; small dense head
  runs redundantly on all cores; core 0's output is returned.

All index/selection data is computed on the host from edge_index/batch at
build time (the Bass graph is compiled after seeing the inputs), but all
feature compute runs on device.
"""
import sys
sys.path.insert(0, "/opt/trn_rl_repo")

import numpy as np
import ml_dtypes

BF16 = ml_dtypes.bfloat16
F8 = ml_dtypes.float8_e4m3

N_NODES, N_EDGES, N_FEAT, HID, HID1, N_GRAPHS, N_CONV, N_LIN = (
    50000, 150000, 128, 512, 320, 2048, 4, 3)
EPS = 1e-5
NC_ = 8
SHARD = N_NODES // NC_          # 6250
BLKS = 49
SLOTS = BLKS * 128              # 6272
CAB = 24                        # blocks in chunk A (groups 0-5)
CBB = BLKS - CAB                # 25 blocks in chunk B (groups 6-12)
CAS = CAB * 128                 # 3072 slots
CBS = CBB * 128                 # 3200 slots
PG = 384                        # pooling window width (3*128)
GRP = 4                         # blocks per gather/dense group
F1P = 384                       # HID1 padded to 3*128
AG_A_EMIT = 9                   # emit AG-A trigger after this group's gathers

# ---------------------------------------------------------------- host planning


def _pack2(degA, degB, nblk, capA, capB):
    """FFD-pack len(degA) nodes into nblk blocks of <=128 nodes s.t. per
    block sum(degA) <= capA and sum(degB) <= capB. Returns slot index
    (block*128+pos) or None."""
    n = len(degA)
    order = np.argsort(-(degA + degB))
    blk_cnt = np.zeros(nblk, np.int32)
    bA = np.zeros(nblk, np.int64)
    bB = np.zeros(nblk, np.int64)
    assign = np.full(n, -1, np.int32)
    for node in order:
        a, b2 = degA[node], degB[node]
        ok = (blk_cnt < 128) & (bA + a <= capA) & (bB + b2 <= capB)
        if not ok.any():
            return None
        cand = np.nonzero(ok)[0]
        j = cand[np.argmin(bA[cand] + bB[cand])]
        assign[node] = j
        blk_cnt[j] += 1
        bA[j] += a
        bB[j] += b2
    slot = np.full(n, -1, np.int32)
    nxt = np.zeros(nblk, np.int32)
    for node in range(n):
        j = assign[node]
        slot[node] = j * 128 + nxt[j]
        nxt[j] += 1
    return slot


def build_plan(edge_index, batch):
    src = edge_index[0].astype(np.int64)
    dst = edge_index[1].astype(np.int64)
    core_of = np.minimum(np.arange(N_NODES) // SHARD, NC_ - 1)
    deg_tot = np.bincount(dst, minlength=N_NODES)

    # phase 0: pack by total degree to get provisional chunk labels
    TL, TH = 2, 2
    slot0 = np.zeros(N_NODES, np.int64)
    for c in range(NC_):
        nodes = np.arange(c * SHARD, (c + 1) * SHARD)
        t = TL + TH
        while True:
            s = _pack2(deg_tot[nodes], np.zeros(SHARD, np.int64), BLKS,
                       t * 128, 1 << 30)
            if s is not None:
                break
            t += 1
        slot0[nodes] = s
    in_a = slot0 < CAS   # chunk label per node (source side), frozen now

    # per-node degrees toward A/B-sourced edges
    degA_n = np.bincount(dst[in_a[src]], minlength=N_NODES)
    degB_n = np.bincount(dst[~in_a[src]], minlength=N_NODES)

    # phase 1: repack each chunk of each core separately with stream caps
    slot_of = np.zeros(N_NODES, np.int64)
    while True:
        ok = True
        for c in range(NC_):
            nodes = np.arange(c * SHARD, (c + 1) * SHARD)
            la = in_a[nodes]
            na, nb = nodes[la], nodes[~la]
            if len(na) > CAS or len(nb) > CBS:
                raise RuntimeError("chunk overflow %d %d" % (len(na), len(nb)))
            sa = _pack2(degA_n[na], degB_n[na], CAB, TL * 128, TH * 128)
            sb = _pack2(degA_n[nb], degB_n[nb], CBB, TL * 128, TH * 128)
            if sa is None or sb is None:
                ok = False
                break
            slot_of[na] = sa
            slot_of[nb] = CAS + sb
        if ok:
            break
        if TL <= TH:
            TL += 1
        else:
            TH += 1
    NT = TL + TH

    # pool rows (per-chunk index spaces)
    assert CAS * NC_ <= 32768 and CBS * NC_ <= 32768
    prow = np.where(slot_of < CAS,
                    core_of * CAS + slot_of,
                    core_of * CBS + (slot_of - CAS))

    dst_core = core_of[dst]
    dst_slot = slot_of[dst]
    dst_blk = dst_slot // 128
    dst_col = dst_slot % 128
    src_in_a = in_a[src]

    idx_all = np.zeros((NC_, BLKS, NT, 128), np.int16)
    m_all = np.zeros((NC_, BLKS, NT, 128, 128), np.float32)
    snode = np.full((NC_, BLKS, NT, 128), -1, np.int64)
    for c in range(NC_):
        sel = dst_core == c
        e_idx = np.nonzero(sel)[0]
        b_of = dst_blk[e_idx]
        order = np.argsort(b_of, kind="stable")
        e_idx = e_idx[order]
        b_of = b_of[order]
        bounds = np.searchsorted(b_of, np.arange(BLKS + 1))
        for b in range(BLKS):
            es = e_idx[bounds[b]:bounds[b + 1]]
            a_es = es[src_in_a[es]]
            b_es = es[~src_in_a[es]]
            assert len(a_es) <= TL * 128 and len(b_es) <= TH * 128, (c, b)
            for eset, t0 in ((a_es, 0), (b_es, TL)):
                rel = prow[src[eset]]
                t = t0 + np.arange(len(eset)) // 128
                r = np.arange(len(eset)) % 128
                idx_all[c, b, t, r] = rel.astype(np.int16)
                snode[c, b, t, r] = src[eset]
                m_all[c, b, t, r, dst_col[eset]] = 1.0

    deg = np.bincount(dst, minlength=N_NODES).astype(np.float32) + 1.0
    deg_slots = np.zeros((NC_, SLOTS), np.float32)
    deg_slots[core_of, slot_of] = deg

    # pooling
    cnt = np.bincount(batch, minlength=N_GRAPHS).astype(np.float32)
    inv_cnt = (1.0 / np.maximum(cnt, 1.0)).astype(np.float32)
    g_of = batch.astype(np.int64)
    wbase = np.zeros(NC_, np.int32)
    mpool = np.zeros((NC_, BLKS, 128, PG), np.float32)
    for c in range(NC_):
        nodes = np.arange(c * SHARD, (c + 1) * SHARD)
        gmin, gmax = g_of[nodes].min(), g_of[nodes].max()
        wb = min(max(0, (gmin + gmax + 1) // 2 - PG // 2), N_GRAPHS - PG)
        wb = min(wb, gmin)
        wb = max(wb, gmax - PG + 1)
        assert wb >= 0 and wb + PG <= N_GRAPHS and gmin >= wb and gmax < wb + PG, \
            (c, gmin, gmax, wb)
        wbase[c] = wb
        cols = slot_of[nodes] % 128
        blks = slot_of[nodes] // 128
        mpool[c, blks, cols, g_of[nodes] - wb] = 1.0

    return dict(slot_of=slot_of, core_of=core_of,
                TL=TL, TH=TH, idx=idx_all, M=m_all, deg=deg_slots,
                snode=snode, mpool=mpool, wbase=wbase, inv_cnt=inv_cnt)


def fold_params(p):
    out = []
    for l in range(5):
        if l == 0:
            ing, inb, inm, inv = p['in_g1'], p['in_b1'], p['in_m1'], p['in_v1']
            wa, ba, wb, bb = p['w1a'], p['b1a'], p['w1b'], p['b1b']
            og, ob, om, ov = p['out_g1'], p['out_b1'], p['out_m1'], p['out_v1']
        else:
            i = l - 1
            ing, inb, inm, inv = (p['cin_g'][i], p['cin_b'][i],
                                  p['cin_m'][i], p['cin_v'][i])
            wa, ba, wb, bb = p['cwA'][i], p['cbA'][i], p['cwB'][i], p['cbB'][i]
            og, ob, om, ov = (p['cout_g'][i], p['cout_b'][i],
                              p['cout_m'][i], p['cout_v'][i])
        gin = np.asarray(ing / np.sqrt(inv + EPS), np.float64)
        bin_ = np.asarray(inb - inm * gin, np.float64)
        gout = np.asarray(og / np.sqrt(ov + EPS), np.float64)
        bout = np.asarray(ob - om * gout, np.float64)
        WA = np.asarray(wa, np.float64) * gin[:, None]   # BN-in gain folded
        cvec = np.asarray(wa, np.float64).T @ bin_       # [HID1]: deg coeff
        WB = np.asarray(wb, np.float64) * gout[None, :]
        bB = np.asarray(bb, np.float64) * gout + bout
        out.append(dict(WA=np.asarray(WA, np.float32),
                        cvec=np.asarray(cvec, np.float32),
                        bA=np.asarray(ba, np.float32),
                        WB=np.asarray(WB, np.float32),
                        bB=np.asarray(bB, np.float32)))
    return out


# ---------------------------------------------------------------- device build


def build_device(TL, TH):
    """Build the Bacc graph (shapes only; all data arrives via in_maps)."""
    from concourse import bass, bacc, mybir, tile

    NT = TL + TH
    dt = mybir.dt
    nc = bacc.Bacc("TRN2", target_bir_lowering=False, debug=False,
                   enable_asserts=False, num_devices=NC_,
                   num_swdge_queues=4)

    def inp(name, shape, dtype):
        return nc.dram_tensor(name, shape, dtype, kind="ExternalInput")

    x_in = inp("x", [SLOTS, N_FEAT], dt.bfloat16)
    gx_in = inp("gx", [128, BLKS * NT * 128], dt.float8e4)
    gl_in = inp("gl", [128, BLKS * TL * 8], dt.int16)
    gh_in = inp("gh", [128, BLKS * TH * 8], dt.int16)
    m8_in = inp("m8", [128, BLKS * NT * 128], dt.float8e4)
    mp_in = inp("mp", [128, BLKS * PG], dt.float8e4)
    rkw_in = inp("rkw", [2, 5 * 512], dt.bfloat16)
    rkr_in = inp("rkr", [2, SLOTS], dt.bfloat16)
    invc_in = inp("invc", [128, N_GRAPHS], dt.bfloat16)
    pvec_in = inp("pvec", [128, 16], dt.float32)
    brow_in = inp("brow", [1, 5 * 512], dt.bfloat16)
    ones_in = inp("ones", [1, 128], dt.bfloat16)
    ident_in = inp("ident", [128, 128], dt.bfloat16)
    ident8_in = inp("ident8", [128, 128], dt.float8e4)
    wa0_in = inp("wa0", [128, F1P], dt.bfloat16)
    wb0_in = inp("wb0", [128, 3 * 512], dt.bfloat16)
    wa8_in = inp("wa8", [4, 128, 4096], dt.float8e4)
    wb8_in = inp("wb8", [4, 128, 2048], dt.float8e4)
    lw_in = inp("lw", [3, 128, 4 * 512], dt.bfloat16)
    fw_in = inp("fw", [128, 4], dt.bfloat16)
    out_ext = nc.dram_tensor("out", [N_GRAPHS, 1], dt.float32,
                             kind="ExternalOutput")

    # group structure: 12 groups of 4 blocks + 1 group of 1 block
    groups = [list(range(g * GRP, min((g + 1) * GRP, BLKS)))
              for g in range((BLKS + GRP - 1) // GRP)]

    PV_LB = lambda l, chunk: 4 * l + chunk

    with tile.TileContext(nc) as tc:
        import contextlib
        ctx = contextlib.ExitStack()
        with ctx:
            dram = ctx.enter_context(tc.tile_pool(name="dram", bufs=1,
                                                  space="DRAM"))
            const = ctx.enter_context(tc.tile_pool(name="const", bufs=1))

            # DRAM: per-layer chunked activation pools + bounces
            pool_a = [dram.tile([NC_ * CAS, HID], dt.float8e4,
                                addr_space="Shared", name=f"pool_a{i}")
                      for i in range(4)]
            pool_b = [dram.tile([NC_ * CBS, HID], dt.float8e4,
                                addr_space="Shared", name=f"pool_b{i}")
                      for i in range(4)]
            bounce_a = [dram.tile([CAS, HID], dt.float8e4,
                                  name=f"bounce_a{i}") for i in range(4)]
            bounce_b = [dram.tile([CBS, HID], dt.float8e4,
                                  name=f"bounce_b{i}") for i in range(4)]
            win_bounce = dram.tile([4 * 128, PG], dt.bfloat16)
            wins_all = dram.tile([NC_ * 4 * 128, PG], dt.bfloat16,
                                 addr_space="Shared")

            # persistent SBUF
            gl_sb = const.tile([128, BLKS * TL * 8], dt.int16)
            gh_sb = const.tile([128, BLKS * TH * 8], dt.int16)
            ux = const.tile([128, BLKS * N_FEAT], dt.bfloat16)
            u_loc = const.tile([128, BLKS * HID], dt.float8e4)
            rkw = const.tile([2, 5 * 512], dt.bfloat16)
            rkr = const.tile([2, SLOTS], dt.bfloat16)
            invc = const.tile([128, N_GRAPHS], dt.bfloat16)
            pvec = const.tile([128, 16], dt.float32)
            brow = const.tile([1, 5 * 512], dt.bfloat16)
            onesr = const.tile([1, 128], dt.bfloat16)
            ident = const.tile([128, 128], dt.bfloat16)
            ident8 = const.tile([128, 128], dt.float8e4)
            wa0 = const.tile([128, F1P], dt.bfloat16)
            wb0 = const.tile([128, 3 * 512], dt.bfloat16)
            wa8_sb = [const.tile([128, 4096], dt.float8e4, name=f"wa8{i}")
                      for i in range(4)]
            wb8_sb = [const.tile([128, 2048], dt.float8e4, name=f"wb8{i}")
                      for i in range(4)]
            lw_sb = [const.tile([128, 4 * 512], dt.bfloat16, name=f"lwt{i}")
                     for i in range(3)]
            fw_sb = const.tile([128, 4], dt.bfloat16)

            nc.sync.dma_start(out=gl_sb[:], in_=gl_in[:])
            nc.sync.dma_start(out=gh_sb[:], in_=gh_in[:])
            nc.sync.dma_start(out=rkw[:], in_=rkw_in[:])
            nc.sync.dma_start(out=rkr[:], in_=rkr_in[:])
            nc.sync.dma_start(out=invc[:], in_=invc_in[:])
            nc.sync.dma_start(out=pvec[:], in_=pvec_in[:])
            nc.sync.dma_start(out=brow[:], in_=brow_in[:])
            nc.sync.dma_start(out=onesr[:], in_=ones_in[:])
            nc.sync.dma_start(out=ident[:], in_=ident_in[:])
            nc.sync.dma_start(out=ident8[:], in_=ident8_in[:])
            nc.sync.dma_start(out=wa0[:], in_=wa0_in[:])
            nc.sync.dma_start(out=wb0[:], in_=wb0_in[:])
            for i in range(4):
                nc.sync.dma_start(out=wa8_sb[i][:], in_=wa8_in[i])
                nc.sync.dma_start(out=wb8_sb[i][:], in_=wb8_in[i])
            for i in range(3):
                nc.sync.dma_start(out=lw_sb[i][:], in_=lw_in[i])
            nc.sync.dma_start(out=fw_sb[:], in_=fw_in[:])

            nc.sync.dma_start(
                out=ux[:].rearrange("p (b f) -> p b f", b=BLKS),
                in_=x_in[:].rearrange("(b p) f -> p b f", p=128))

            conv_ctx = contextlib.ExitStack()
            gpool = conv_ctx.enter_context(tc.tile_pool(name="gpool", bufs=2))
            mpoolp = conv_ctx.enter_context(tc.tile_pool(name="mpoolp", bufs=2))
            aggp = conv_ctx.enter_context(tc.tile_pool(name="aggp", bufs=8))
            h1p = conv_ctx.enter_context(tc.tile_pool(name="h1p", bufs=8))
            psA = conv_ctx.enter_context(tc.tile_pool(name="psA", bufs=4,
                                                      space="PSUM"))
            psB = conv_ctx.enter_context(tc.tile_pool(name="psB", bufs=2,
                                                      space="PSUM"))
            psC = conv_ctx.enter_context(tc.tile_pool(name="psC", bufs=2,
                                                      space="PSUM"))

            def conv_layer(l, src_a, src_b, u_src, dst_a, dst_b, bnc_a, bnc_b):
                """One sumconv layer. u_src: fp8 (or bf16 for l=0) SBUF tile of
                local activations (selfloop source, [128, BLKS*F_in])."""
                F_in = N_FEAT if l == 0 else HID
                FC = F_in // 128
                F1C = 3 if l == 0 else 4
                id_t = ident if l == 0 else ident8
                for gi, blks in enumerate(groups):
                    nb = len(blks)
                    b0 = blks[0]
                    g_l = gpool.tile([128, GRP * TL, F_in], dt.float8e4,
                                     tag="gl", bufs=4)
                    g_h = gpool.tile([128, GRP * TH, F_in], dt.float8e4,
                                     tag="gh", bufs=4)
                    if l == 0:
                        nc.sync.dma_start(
                            out=g_l[:, :nb * TL, :],
                            in_=gx_in[:, (b0 * NT) * 128:
                                      (b0 * NT + nb * TL) * 128]
                                .rearrange("p (t f) -> p t f", f=F_in))
                        nc.sync.dma_start(
                            out=g_h[:, :nb * TH, :],
                            in_=gx_in[:, (b0 * NT + nb * TL) * 128:
                                      (b0 + nb) * NT * 128]
                                .rearrange("p (t f) -> p t f", f=F_in))
                    else:
                        nc.gpsimd.dma_gather(
                            out_ap=g_l[:, :nb * TL, :],
                            in_ap=src_a[:],
                            idxs_ap=gl_sb[:, b0 * TL * 8:(b0 + nb) * TL * 8],
                            num_idxs=nb * TL * 128,
                            num_idxs_reg=nb * TL * 128,
                            elem_size=F_in, single_packet=False,
                            queue_num=(gi % 2) * 2)
                        nc.gpsimd.dma_gather(
                            out_ap=g_h[:, :nb * TH, :],
                            in_ap=src_b[:],
                            idxs_ap=gh_sb[:, b0 * TH * 8:(b0 + nb) * TH * 8],
                            num_idxs=nb * TH * 128,
                            num_idxs_reg=nb * TH * 128,
                            elem_size=F_in, single_packet=False,
                            queue_num=(gi % 2) * 2 + 1)

                    # AG-A trigger for THIS layer's output: emitted late in
                    # the gather FIFO so it doesn't stall it; fires once
                    # bounce_a (groups 0-5) lands.
                    if gi == AG_A_EMIT and dst_a is not None:
                        nc.gpsimd.collective_compute(
                            "AllGather", mybir.AluOpType.bypass,
                            replica_groups=[list(range(NC_))],
                            ins=[bnc_a[:]], outs=[dst_a[:]])

                    m_sb = mpoolp.tile([128, nb * NT * 128], dt.float8e4,
                                       tag="m", bufs=2)
                    nc.sync.dma_start(
                        out=m_sb[:],
                        in_=m8_in[:, b0 * NT * 128:(b0 + nb) * NT * 128])

                    # aggregation into PSUM, DoubleRow over stream tile pairs
                    agg_ps = [psA.tile([128, 512], dt.float32, tag="aggps",
                                       name=f"aggps{fc}", bufs=4)
                              for fc in range(FC)]
                    for bi, b in enumerate(blks):
                        for fc in range(FC):
                            o = agg_ps[fc][:, bi * 128:(bi + 1) * 128]
                            first = [True]

                            def stream_mms(gt, tbase, t0, ntile):
                                t = 0
                                while t < ntile:
                                    if t + 2 <= ntile:
                                        nc.tensor.matmul(
                                            out=o,
                                            lhsT=gt[:, tbase + t:tbase + t + 2,
                                                    fc * 128:(fc + 1) * 128],
                                            rhs=m_sb[:, (bi * NT + t0 + t) * 128:
                                                     (bi * NT + t0 + t + 2) * 128]
                                                .rearrange("p (u d) -> p u d",
                                                           u=2),
                                            start=first[0], stop=False,
                                            perf_mode=(
                                                mybir.MatmulPerfMode.DoubleRow),
                                        )
                                        t += 2
                                    else:
                                        nc.tensor.matmul(
                                            out=o,
                                            lhsT=gt[:, tbase + t,
                                                    fc * 128:(fc + 1) * 128],
                                            rhs=m_sb[:, (bi * NT + t0 + t) * 128:
                                                     (bi * NT + t0 + t + 1) * 128],
                                            start=first[0], stop=False)
                                        t += 1
                                    first[0] = False

                            stream_mms(g_l, bi * TL, 0, TL)
                            stream_mms(g_h, bi * TH, TL, TH)
                            # self loop (raw activations)
                            nc.tensor.matmul(
                                out=o,
                                lhsT=u_src[:, b * F_in + fc * 128:
                                           b * F_in + (fc + 1) * 128],
                                rhs=id_t[:], start=False, stop=True)

                    w = nb * 128
                    if l == 0:
                        # ---- layer 0: bf16 dense path
                        agg_sb = [aggp.tile([128, 512], dt.bfloat16, tag="agg",
                                            name=f"aggsb{fc}", bufs=8)
                                  for fc in range(FC)]
                        for fc in range(FC):
                            nc.vector.tensor_copy(
                                out=agg_sb[fc][:, :w], in_=agg_ps[fc][:, :w])
                        h1_sb = [h1p.tile([128, 512], dt.bfloat16, tag="h1",
                                          name=f"h1sb{m}", bufs=8)
                                 for m in range(F1C)]
                        for m in range(F1C):
                            h1_ps = psB.tile([128, 512], dt.float32,
                                             tag="h1ps")
                            for fc in range(FC):
                                nc.tensor.matmul(
                                    out=h1_ps[:, :w],
                                    lhsT=wa0[:, fc * F1P + m * 128:
                                             fc * F1P + (m + 1) * 128],
                                    rhs=agg_sb[fc][:, :w],
                                    start=(fc == 0), stop=False)
                            nc.tensor.matmul(
                                out=h1_ps[:, :w],
                                lhsT=rkw[:, m * 128:(m + 1) * 128],
                                rhs=rkr[:, b0 * 128:b0 * 128 + w],
                                start=False, stop=True)
                            nc.scalar.activation(
                                out=h1_sb[m][:, :w], in_=h1_ps[:, :w],
                                func=mybir.ActivationFunctionType.Relu)
                        for bi, b in enumerate(blks):
                            h2_ps = psC.tile([128, 512], dt.float32,
                                             tag="h2ps")
                            for k in range(F1C):
                                nc.tensor.matmul(
                                    out=h2_ps[:],
                                    lhsT=h1_sb[k][:, bi * 128:(bi + 1) * 128],
                                    rhs=wb0[:, k * 512:(k + 1) * 512],
                                    start=(k == 0), stop=False)
                            nc.tensor.matmul(
                                out=h2_ps[:],
                                lhsT=onesr[:],
                                rhs=brow[:, 0:512],
                                start=False, stop=True)
                            nc.scalar.activation(
                                out=u_loc[:, b * HID:(b + 1) * HID],
                                in_=h2_ps[:],
                                func=mybir.ActivationFunctionType.Relu)
                    else:
                        # ---- layers 1-4: fp8 DoubleRow dense path (x64
                        # weight scaling, descaled in the relu)
                        agg8 = aggp.tile([128, FC, 512], dt.float8e4,
                                         tag="agg", bufs=8)
                        for fc in range(FC):
                            nc.vector.tensor_copy(
                                out=agg8[:, fc, :w], in_=agg_ps[fc][:, :w])
                        h1_all = h1p.tile([128, F1C, 512], dt.float8e4,
                                          tag="h1", bufs=8)
                        for m in range(F1C):
                            h1_ps = psB.tile([128, 512], dt.float32,
                                             tag="h1ps")
                            for p in range(2):
                                nc.tensor.matmul(
                                    out=h1_ps[:, :w],
                                    lhsT=wa8_sb[l - 1][
                                        :, ((p * 4 + m) * 2) * 128:
                                           ((p * 4 + m) * 2 + 2) * 128]
                                        .rearrange("q (o j) -> q o j", o=2),
                                    rhs=agg8[:, 2 * p:2 * p + 2, :w],
                                    start=(p == 0), stop=False,
                                    perf_mode=mybir.MatmulPerfMode.DoubleRow)
                            nc.tensor.matmul(
                                out=h1_ps[:, :w],
                                lhsT=rkw[:, l * 512 + m * 128:
                                         l * 512 + (m + 1) * 128],
                                rhs=rkr[:, b0 * 128:b0 * 128 + w],
                                start=False, stop=True)
                            nc.scalar.activation(
                                out=h1_all[:, m, :w], in_=h1_ps[:, :w],
                                func=mybir.ActivationFunctionType.Relu,
                                scale=1.0 / 64.0)
                        for bi, b in enumerate(blks):
                            h2_ps = psC.tile([128, 512], dt.float32,
                                             tag="h2ps")
                            for q in range(2):
                                nc.tensor.matmul(
                                    out=h2_ps[:],
                                    lhsT=h1_all[:, 2 * q:2 * q + 2,
                                                bi * 128:(bi + 1) * 128],
                                    rhs=wb8_sb[l - 1][
                                        :, (2 * q) * 512:(2 * q + 2) * 512]
                                        .rearrange("p (o j) -> p o j", o=2),
                                    start=(q == 0), stop=False,
                                    perf_mode=mybir.MatmulPerfMode.DoubleRow)
                            nc.tensor.matmul(
                                out=h2_ps[:],
                                lhsT=onesr[:],
                                rhs=brow[:, l * 512:(l + 1) * 512],
                                start=False, stop=True)
                            nc.scalar.activation(
                                out=u_loc[:, b * HID:(b + 1) * HID],
                                in_=h2_ps[:],
                                func=mybir.ActivationFunctionType.Relu,
                                scale=1.0 / 64.0)

                    if bnc_a is not None:
                        if b0 < CAB:  # groups 0-5 -> chunk A bounce
                            nc.sync.dma_start(
                                out=bnc_a[b0 * 128:(b0 + nb) * 128, :]
                                    .rearrange("(b p) f -> p b f", p=128),
                                in_=u_loc[:, b0 * HID:(b0 + nb) * HID]
                                    .rearrange("p (b f) -> p b f", b=nb))
                        else:
                            c0 = b0 - CAB
                            nc.sync.dma_start(
                                out=bnc_b[c0 * 128:(c0 + nb) * 128, :]
                                    .rearrange("(b p) f -> p b f", p=128),
                                in_=u_loc[:, b0 * HID:(b0 + nb) * HID]
                                    .rearrange("p (b f) -> p b f", b=nb))
                if dst_b is not None:
                    nc.gpsimd.collective_compute(
                        "AllGather", mybir.AluOpType.bypass,
                        replica_groups=[list(range(NC_))],
                        ins=[bnc_b[:]], outs=[dst_b[:]])

            # layer 0 (input conv, gx pre-gathered): writes pools 0
            with nc.named_scope("layer0"):
                conv_layer(0, None, None, ux, pool_a[0], pool_b[0],
                           bounce_a[0], bounce_b[0])
            for l in range(1, 5):
                sa, sb2 = pool_a[l - 1], pool_b[l - 1]
                da = pool_a[l] if l < 4 else None
                db = pool_b[l] if l < 4 else None
                ba2 = bounce_a[l] if l < 4 else None
                bb2 = bounce_b[l] if l < 4 else None
                with nc.named_scope(f"layer{l}"):
                    conv_layer(l, sa, sb2, u_loc, da, db, ba2, bb2)
            conv_ctx.close()

            # ---------------- pooling into per-core graph window
            with tc.tile_pool(name="pps", bufs=4, space="PSUM") as pps, \
                 tc.tile_pool(name="mpp", bufs=2) as mpp, \
                 tc.tile_pool(name="winp", bufs=1) as winp:
                pool_ps = [pps.tile([128, PG], dt.float32, name=f"poolps{fc}",
                                    tag="poolps", bufs=4)
                           for fc in range(4)]
                for b in range(BLKS):
                    mp_sb = mpp.tile([128, PG], dt.float8e4, tag="mp")
                    nc.sync.dma_start(out=mp_sb[:],
                                      in_=mp_in[:, b * PG:(b + 1) * PG])
                    for fc in range(4):
                        nc.tensor.matmul(
                            out=pool_ps[fc][:],
                            lhsT=u_loc[:, b * HID + fc * 128:
                                       b * HID + (fc + 1) * 128],
                            rhs=mp_sb[:],
                            start=(b == 0), stop=(b == BLKS - 1))
                win_sb = winp.tile([128, 4 * PG], dt.bfloat16)
                for fc in range(4):
                    nc.vector.tensor_copy(
                        out=win_sb[:, fc * PG:(fc + 1) * PG],
                        in_=pool_ps[fc][:])
                nc.sync.dma_start(
                    out=win_bounce[:].rearrange("(c p) g -> p c g", p=128),
                    in_=win_sb[:].rearrange("p (c g) -> p c g", c=4))
            nc.gpsimd.collective_compute(
                "AllGather", mybir.AluOpType.bypass,
                replica_groups=[list(range(NC_))],
                ins=[win_bounce[:]], outs=[wins_all[:]])

            # ---------------- reconstruction + head (redundant on all cores)
            with tc.tile_pool(name="headp", bufs=1) as hp, \
                 tc.tile_pool(name="wtmpp", bufs=4) as wtp, \
                 tc.tile_pool(name="hps", bufs=4, space="PSUM") as hps:
                pool_full = hp.tile([128, 4 * N_GRAPHS], dt.bfloat16)
                nc.vector.memset(pool_full[:], 0)
                for w in range(NC_):
                    wtmp = wtp.tile([128, 4 * PG], dt.bfloat16, tag="wtmp")
                    nc.sync.dma_start(
                        out=wtmp[:].rearrange("p (c g) -> p c g", c=4),
                        in_=wins_all[w * 512:(w + 1) * 512, :]
                            .rearrange("(c p) g -> p c g", p=128))
                    for fc in range(4):
                        dstv = pool_full[:, fc * N_GRAPHS + WBASES[w]:
                                         fc * N_GRAPHS + WBASES[w] + PG]
                        nc.vector.tensor_add(
                            out=dstv, in0=dstv,
                            in1=wtmp[:, fc * PG:(fc + 1) * PG])
                # mean-pool normalization (sums -> means)
                for fc in range(4):
                    nc.vector.tensor_tensor(
                        out=pool_full[:, fc * N_GRAPHS:(fc + 1) * N_GRAPHS],
                        in0=pool_full[:, fc * N_GRAPHS:(fc + 1) * N_GRAPHS],
                        in1=invc[:],
                        op=mybir.AluOpType.mult)

                cur = pool_full
                for li in range(3):
                    nxt = hp.tile([128, 4 * N_GRAPHS], dt.bfloat16,
                                  name=f"head{li}", tag="headbuf", bufs=2)
                    for nk in range(4):
                        for m in range(4):
                            ps = hps.tile([128, 512], dt.float32, tag="hps")
                            for k in range(4):
                                nc.tensor.matmul(
                                    out=ps[:],
                                    lhsT=lw_sb[li][:, k * 512 + m * 128:
                                                   k * 512 + (m + 1) * 128],
                                    rhs=cur[:, k * N_GRAPHS + nk * 512:
                                            k * N_GRAPHS + (nk + 1) * 512],
                                    start=(k == 0), stop=(k == 3))
                            nc.scalar.activation(
                                out=nxt[:, m * N_GRAPHS + nk * 512:
                                        m * N_GRAPHS + (nk + 1) * 512],
                                in_=ps[:],
                                func=mybir.ActivationFunctionType.Relu,
                                bias=pvec[:, PV_LB(li, m):PV_LB(li, m) + 1])
                    cur = nxt
                osb = hp.tile([1, N_GRAPHS], dt.float32)
                for nk in range(4):
                    ps = hps.tile([1, 512], dt.float32, tag="ops")
                    for k in range(4):
                        nc.tensor.matmul(
                            out=ps[:],
                            lhsT=fw_sb[:, k:k + 1],
                            rhs=cur[:, k * N_GRAPHS + nk * 512:
                                    k * N_GRAPHS + (nk + 1) * 512],
                            start=(k == 0), stop=(k == 3))
                    nc.scalar.activation(
                        out=osb[:, nk * 512:(nk + 1) * 512], in_=ps[:],
                        func=mybir.ActivationFunctionType.Copy, bias=FB_CONST)
                nc.sync.dma_start(
                    out=out_ext[:].rearrange("g one -> one g"),
                    in_=osb[:])
    nc.compile()
    return nc


# WBASES / FB_CONST are module-level so build_device can see them; set in kernel()
WBASES = None
FB_CONST = 0.0


# ---------------------------------------------------------------- host packing


def make_in_maps(inputs, plan, layers):
    TL, TH = plan["TL"], plan["TH"]
    NT = TL + TH
    slot_of, core_of = plan["slot_of"], plan["core_of"]
    x = np.asarray(inputs["x"], np.float32)
    x8 = x.astype(F8).astype(np.float32)

    def wrap_idx(flat):
        """[N] int16 gather positions -> [128, N/16] wrapped+replicated."""
        n = len(flat)
        arr = flat.reshape(n // 16, 16).T.astype(np.int16)  # [16, n/16]
        return np.tile(arr, (8, 1))

    in_maps = []
    for c in range(NC_):
        m = {}
        xs = np.zeros((SLOTS, N_FEAT), np.float32)
        nodes = np.arange(c * SHARD, (c + 1) * SHARD)
        xs[slot_of[nodes]] = x[nodes]
        m["x"] = xs.astype(BF16)

        gl = plan["idx"][c, :, :TL, :].reshape(-1)
        gh = plan["idx"][c, :, TL:, :].reshape(-1)
        m["gl"] = wrap_idx(gl)
        m["gh"] = wrap_idx(gh)

        # layer-0 pre-gathered G, span-grouped to match device consumption:
        # per span of blocks: A-tiles (block-major, t<TL) then B-tiles.
        sn = plan["snode"][c]                            # [BLKS, NT, 128]
        G0 = np.where(sn[..., None] >= 0,
                      x8[np.maximum(sn, 0)], 0.0)        # [BLKS, NT, 128, F]
        gx = np.zeros((128, BLKS * NT * 128), np.float32)
        col = 0
        b0s = 0
        while b0s < BLKS:
            nbg = min(GRP, BLKS - b0s)
            for b in range(b0s, b0s + nbg):
                for t in range(TL):
                    gx[:, col:col + N_FEAT] = G0[b, t]
                    col += N_FEAT
            for b in range(b0s, b0s + nbg):
                for t in range(TL, NT):
                    gx[:, col:col + N_FEAT] = G0[b, t]
                    col += N_FEAT
            b0s += nbg
        m["gx"] = gx.astype(F8)

        mt = plan["M"][c].reshape(BLKS * NT, 128, 128)
        msw = np.ascontiguousarray(mt.transpose(1, 0, 2).reshape(128, -1))
        m["m8"] = msw.astype(F8)

        mp = plan["mpool"][c]                            # [BLKS, 128, PG]
        m["mp"] = np.ascontiguousarray(
            mp.transpose(1, 0, 2).reshape(128, -1)).astype(F8)

        rkw = np.zeros((2, 5 * 512), np.float32)
        for l in range(5):
            L = layers[l]
            s = 1.0 if l == 0 else 64.0                  # match fp8 weight scale
            n1 = len(L["cvec"])                          # 320 or 512
            rkw[0, l * 512:l * 512 + n1] = L["cvec"] * s
            rkw[1, l * 512:l * 512 + n1] = L["bA"] * s
        m["rkw"] = rkw.astype(BF16)

        rkr = np.zeros((2, SLOTS), np.float32)
        rkr[0] = plan["deg"][c]
        rkr[1] = 1.0
        m["rkr"] = rkr.astype(BF16)

        m["invc"] = np.tile(plan["inv_cnt"][None, :], (128, 1)).astype(BF16)

        pvec = np.zeros((128, 16), np.float32)
        for li in range(3):
            lb = np.asarray(inputs["lb"][li], np.float32)
            for mm in range(4):
                pvec[:, 4 * li + mm] = lb[mm * 128:(mm + 1) * 128]
        m["pvec"] = pvec

        brow = np.zeros((1, 5 * 512), np.float32)
        for l in range(5):
            s = 1.0 if l == 0 else 64.0
            brow[0, l * 512:(l + 1) * 512] = layers[l]["bB"] * s
        m["brow"] = brow.astype(BF16)

        m["ones"] = np.ones((1, 128), np.float32).astype(BF16)
        m["ident"] = np.eye(128, dtype=np.float32).astype(BF16)
        m["ident8"] = np.eye(128, dtype=np.float32).astype(F8)

        wa0 = np.zeros((128, F1P), np.float32)
        wa0[:, :HID1] = layers[0]["WA"]
        m["wa0"] = wa0.astype(BF16)
        wb0 = np.zeros((128, 3 * 512), np.float32)
        WB0 = layers[0]["WB"]
        for k in range(3):
            seg = WB0[k * 128:(k + 1) * 128]
            wb0[:seg.shape[0], k * 512:(k + 1) * 512] = seg
        m["wb0"] = wb0.astype(BF16)

        wa8 = np.zeros((4, 128, 4096), np.float32)
        wb8 = np.zeros((4, 128, 2048), np.float32)
        for l in range(1, 5):
            WA, WBm = layers[l]["WA"], layers[l]["WB"]
            for p in range(2):
                for mm in range(4):
                    for o in range(2):
                        cb = ((p * 4 + mm) * 2 + o) * 128
                        wa8[l - 1, :, cb:cb + 128] = \
                            64.0 * WA[(2 * p + o) * 128:(2 * p + o + 1) * 128,
                                      mm * 128:(mm + 1) * 128]
            for k in range(4):
                wb8[l - 1, :, k * 512:(k + 1) * 512] = \
                    64.0 * WBm[k * 128:(k + 1) * 128, :]
        m["wa8"] = wa8.astype(F8)
        m["wb8"] = wb8.astype(F8)

        lw = np.zeros((3, 128, 4 * 512), np.float32)
        for li in range(3):
            LW = np.asarray(inputs["lw"][li], np.float32)
            for k in range(4):
                for mm in range(4):
                    lw[li, :, k * 512 + mm * 128:k * 512 + (mm + 1) * 128] = \
                        LW[k * 128:(k + 1) * 128, mm * 128:(mm + 1) * 128]
        m["lw"] = lw.astype(BF16)

        fw = np.zeros((128, 4), np.float32)
        FW = np.asarray(inputs["fw"], np.float32)
        for k in range(4):
            fw[:, k] = FW[k * 128:(k + 1) * 128, 0]
        m["fw"] = fw.astype(BF16)

        in_maps.append(m)
    return in_maps


_CACHE = {}


def kernel(**inputs):
    global WBASES, FB_CONST
    from concourse.bass_utils import run_bass_kernel_spmd

    plan = build_plan(np.asarray(inputs["edge_index"]),
                      np.asarray(inputs["batch"]))
    layers = fold_params({k: np.asarray(v) for k, v in inputs.items()
                          if k not in ("x", "edge_index", "batch")})
    WBASES = [int(v) for v in plan["wbase"]]
    FB_CONST = float(np.asarray(inputs["fb"]).reshape(-1)[0])

    key = (plan["TL"], plan["TH"], tuple(WBASES), FB_CONST)
    if key not in _CACHE:
        _CACHE[key] = build_device(plan["TL"], plan["TH"])
    nc = _CACHE[key]

    in_maps = make_in_maps(inputs, plan, layers)
    res = run_bass_kernel_spmd(nc, in_maps, core_ids=list(range(NC_)),
                               trace=False)
    out = res.results[0]["out"].astype(np.float32)
    return out
